# revision 1
# baseline (speedup 1.0000x reference)
"""MoE routing layer on 8 Trainium2 NeuronCores (data-parallel over batch).

Per core (4 samples):
  routing MLP -> cosine sim vs embeddings -> softmax weights wf[4,10]
  w_eff[b] = sum_n wf[b,n] * conv_w[n]  (conv is linear in weights ->
  10x fewer conv FLOPs than materializing all expert convs)
  out[b] = conv2d(x[b], w_eff[b]) + b_eff[b]

Conv is 9 shifted matmuls over the flat 58-wide grid (tap = constant
free-dim offset); two samples run concurrently on the PE array via
row tiling (partitions 0-63 / 64-127), fp32r for full-rate streaming.
"""
import sys

sys.path.insert(0, "/opt/trn_rl_repo")

import numpy as np

import concourse.bass as bass
import concourse.mybir as mybir
from concourse.masks import make_identity
from concourse.tile import TileContext

F32 = mybir.dt.float32
F32R = mybir.dt.float32r
AF = mybir.ActivationFunctionType
ALU = mybir.AluOpType
AX = mybir.AxisListType

NCORES = 8
BLOC = 4           # samples per core
CIN = 64
COUT = 64
H = W = 58
HW = H * W         # 3364
OH = OW = 56
NB = 10            # experts
EDIM = 64
RSIZE = 512
HID = 128
NTAP = 9
CHUNK_ROWS = 8
NCHUNK = 7         # 7*8 = 56 output rows
NFREE = CHUNK_ROWS * W  # 464 <= 512 (one PSUM bank)
TAP_OFF = [dy * W + dx for dy in range(3) for dx in range(3)]
PAIRED = True      # 2-sample row-tiled PE packing
CONV_DT = F32R


def fix_sync_waits(nc, cap=2):
    """This walrus build allows at most `cap` sem waits per instruction.
    Splice same-engine NoOps carrying the excess waits right before any
    over-subscribed instruction (waits happen earlier => same semantics)."""
    uid = [0]
    for f in nc.m.functions:
        for blk in f.blocks:
            insts = blk.instructions  # live list
            i = 0
            while i < len(insts):
                inst = insts[i]
                si = inst.sync_info
                waits = list(si.on_wait) if si and si.on_wait else []
                icap = 1
                if len(waits) <= icap:
                    i += 1
                    continue
                keep, excess = waits[-icap:], waits[:-icap]
                for k in range(0, len(excess), icap):
                    nop = mybir.InstNoOp(
                        name=f"{inst.name}-wsplit{uid[0]}", ins=[], outs=[]
                    )
                    uid[0] += 1
                    nop.engine = inst.engine
                    nop.sync_info = mybir.SyncInfo(
                        on_wait=excess[k : k + icap], on_update=[]
                    )
                    nc.register_instruction(nop, overwrite=True)
                    insts.insert(i, nop)
                    i += 1
                inst.sync_info = mybir.SyncInfo(
                    on_wait=keep,
                    on_update=list(si.on_update) if si and si.on_update else [],
                )
                i += 1


def build():
    nc = bass.Bass(num_swdge_queues=4)
    x = nc.dram_tensor("x", [BLOC, CIN, H, W], F32, kind="ExternalInput")
    rv = nc.dram_tensor("rv", [BLOC, RSIZE], F32, kind="ExternalInput")
    w1 = nc.dram_tensor("w1", [RSIZE, HID], F32, kind="ExternalInput")
    bias1 = nc.dram_tensor("bias1", [HID, 1], F32, kind="ExternalInput")
    w2 = nc.dram_tensor("w2", [HID, EDIM], F32, kind="ExternalInput")
    bias2 = nc.dram_tensor("bias2", [EDIM, 1], F32, kind="ExternalInput")
    emb = nc.dram_tensor("emb", [NB, EDIM], F32, kind="ExternalInput")
    cwp = nc.dram_tensor("cwp", [CIN, NB, NTAP, COUT], F32, kind="ExternalInput")
    cb = nc.dram_tensor("cb", [NB, COUT], F32, kind="ExternalInput")
    sel = nc.dram_tensor("sel", [2, BLOC, 128], F32, kind="ExternalInput")
    identin = nc.dram_tensor("identin", [128, 128], F32, kind="ExternalInput")
    out = nc.dram_tensor("out", [BLOC, COUT, OH, OW], F32, kind="ExternalOutput")

    with TileContext(nc) as tc:
        with (
            tc.tile_pool(name="consts", bufs=1) as consts,
            tc.tile_pool(name="work", bufs=2) as work,
            tc.tile_pool(name="stage", bufs=4) as stage,
            tc.tile_pool(name="ps", bufs=2, space="PSUM") as pspool,
            tc.tile_pool(name="psconv", bufs=2, space="PSUM") as psconv,
        ):
            # ---------- inputs / constants into SBUF ----------
            ident = consts.tile([128, 128], F32, tag="ident")
            nc.sync.dma_start(out=ident[:], in_=identin[:])
            ones64 = consts.tile([EDIM, 1], F32, tag="ones64")
            nc.vector.memset(ones64[:], 1.0)

            rvsb = consts.tile([BLOC, RSIZE], F32, tag="rvsb")
            nc.sync.dma_start(out=rvsb[:], in_=rv[:])
            w1sb = consts.tile([128, 4, HID], F32, tag="w1sb")
            nc.sync.dma_start(
                out=w1sb[:], in_=w1[:].rearrange("(c k) m -> k c m", k=128)
            )
            w2sb = consts.tile([HID, EDIM], F32, tag="w2sb")
            nc.sync.dma_start(out=w2sb[:], in_=w2[:])
            b1sb = consts.tile([HID, 1], F32, tag="b1sb")
            nc.sync.dma_start(out=b1sb[:], in_=bias1[:])
            b2sb = consts.tile([EDIM, 1], F32, tag="b2sb")
            nc.sync.dma_start(out=b2sb[:], in_=bias2[:])
            embsb = consts.tile([NB, EDIM], F32, tag="embsb")
            nc.sync.dma_start(out=embsb[:], in_=emb[:])
            cbsb = consts.tile([NB, COUT], F32, tag="cbsb")
            nc.sync.dma_start(out=cbsb[:], in_=cb[:])
            selsb = consts.tile([BLOC, 2, 128], F32, tag="selsb")
            nc.sync.dma_start(out=selsb[:], in_=sel[:].rearrange("j b p -> b j p"))

            cwp2 = consts.tile([128, NB, NTAP, COUT], F32, tag="cwp2")
            nc.sync.dma_start(out=cwp2[0:64], in_=cwp[:])
            nc.sync.dma_start(out=cwp2[64:128], in_=cwp[:])

            xt = []
            for j in range(2):
                t = consts.tile([128, HW + 4], CONV_DT, tag=f"xt{j}")
                nc.vector.memset(t[:, HW : HW + 4].bitcast(F32), 0.0)
                nc.gpsimd.dma_start(
                    out=t[0:64, 0:HW], in_=x[2 * j].rearrange("c h w -> c (h w)")
                )
                nc.gpsimd.dma_start(
                    out=t[64:128, 0:HW],
                    in_=x[2 * j + 1].rearrange("c h w -> c (h w)"),
                )
                xt.append(t)

            # ---------- routing MLP ----------
            # rv [4, 512] -> rvT [128, 4(chunk), 4(sample)] via PE transposes
            rvT = work.tile([128, 4, BLOC], F32, tag="rvT")
            for c in range(4):
                pst = pspool.tile([128, BLOC], F32, tag="small")
                nc.tensor.transpose(
                    pst[:], rvsb[:, c * 128 : (c + 1) * 128], ident[0:BLOC, 0:BLOC]
                )
                nc.scalar.copy(out=rvT[:, c, :], in_=pst[:])
            h1 = pspool.tile([HID, BLOC], F32, tag="small")
            for c in range(4):
                nc.tensor.matmul(
                    h1[:], w1sb[:, c, :], rvT[:, c, :], start=(c == 0), stop=(c == 3)
                )
            h1r = work.tile([HID, BLOC], F32, tag="h1r")
            nc.scalar.activation(
                out=h1r[:], in_=h1[:], func=AF.Relu, bias=b1sb[:], scale=1.0
            )
            rps = pspool.tile([EDIM, BLOC], F32, tag="small")
            nc.tensor.matmul(rps[:], w2sb[:], h1r[:], start=True, stop=True)
            rsb = work.tile([EDIM, BLOC], F32, tag="rsb")
            nc.scalar.activation(
                out=rsb[:], in_=rps[:], func=AF.Identity, bias=b2sb[:], scale=1.0
            )

            # ---------- cosine similarity ----------
            rsq = work.tile([EDIM, BLOC], F32, tag="rsq")
            nc.vector.tensor_mul(rsq[:], rsb[:], rsb[:])
            nsq = pspool.tile([BLOC, 1], F32, tag="small")
            nc.tensor.matmul(nsq[:], rsq[:], ones64[:], start=True, stop=True)
            rln = work.tile([BLOC, 1], F32, tag="rln")
            nc.scalar.activation(out=rln[:], in_=nsq[:], func=AF.Ln)
            rinv = work.tile([BLOC, 1], F32, tag="rinv")
            nc.scalar.activation(out=rinv[:], in_=rln[:], func=AF.Exp, scale=-0.5)

            esq = work.tile([NB, EDIM], F32, tag="esq")
            nc.vector.tensor_mul(esq[:], embsb[:], embsb[:])
            ensq = work.tile([NB, 1], F32, tag="ensq")
            nc.vector.tensor_reduce(ensq[:], esq[:], axis=AX.X, op=ALU.add)
            eln = work.tile([NB, 1], F32, tag="eln")
            nc.scalar.activation(out=eln[:], in_=ensq[:], func=AF.Ln)
            einv = work.tile([NB, 1], F32, tag="einv")
            nc.scalar.activation(out=einv[:], in_=eln[:], func=AF.Exp, scale=-0.5)
            embn = work.tile([NB, EDIM], F32, tag="embn")
            nc.vector.tensor_scalar_mul(out=embn[:], in0=embsb[:], scalar1=einv[:])
            embnT_ps = pspool.tile([EDIM, NB], F32, tag="small")
            nc.tensor.transpose(embnT_ps[:], embn[:], ident[0:NB, 0:NB])
            embnT = work.tile([EDIM, NB], F32, tag="embnT")
            nc.scalar.copy(out=embnT[:], in_=embnT_ps[:])

            simps = pspool.tile([BLOC, NB], F32, tag="small")
            nc.tensor.matmul(simps[:], rsb[:], embnT[:], start=True, stop=True)
            sim = work.tile([BLOC, NB], F32, tag="sim")
            nc.vector.tensor_scalar_mul(out=sim[:], in0=simps[:], scalar1=rinv[:])

            # ---------- softmax ----------
            mx = work.tile([BLOC, 1], F32, tag="mx")
            nc.vector.tensor_reduce(mx[:], sim[:], axis=AX.X, op=ALU.max)
            negmx = work.tile([BLOC, 1], F32, tag="negmx")
            nc.vector.tensor_scalar_mul(out=negmx[:], in0=mx[:], scalar1=-1.0)
            ex = work.tile([BLOC, NB], F32, tag="ex")
            nc.scalar.activation(
                out=ex[:], in_=sim[:], func=AF.Exp, bias=negmx[:], scale=1.0
            )
            s = work.tile([BLOC, 1], F32, tag="s")
            nc.vector.tensor_reduce(s[:], ex[:], axis=AX.X, op=ALU.add)
            sinv = work.tile([BLOC, 1], F32, tag="sinv")
            nc.vector.reciprocal(sinv[:], s[:])
            wf = work.tile([BLOC, NB], F32, tag="wf")
            nc.vector.tensor_scalar_mul(out=wf[:], in0=ex[:], scalar1=sinv[:])

            # ---------- effective conv bias ----------
            wfT_ps = pspool.tile([NB, BLOC], F32, tag="small")
            nc.tensor.transpose(wfT_ps[:], wf[:], ident[0:BLOC, 0:BLOC])
            wfT = work.tile([NB, BLOC], F32, tag="wfT")
            nc.scalar.copy(out=wfT[:], in_=wfT_ps[:])
            beff_ps = pspool.tile([COUT, BLOC], F32, tag="small")
            nc.tensor.matmul(beff_ps[:], cbsb[:], wfT[:], start=True, stop=True)
            beff = work.tile([COUT, BLOC], F32, tag="beff")
            nc.scalar.copy(out=beff[:], in_=beff_ps[:])

            # ---------- PE warmup: keep HAM busy until conv starts ----------
            warm_ps = pspool.tile([128, 512], F32, tag="warm")
            wl = ident[:].bitcast(mybir.dt.bfloat16)[:, 0:128]
            wr = w1sb[:].rearrange("p c m -> p (c m)").bitcast(mybir.dt.bfloat16)[:, 0:512]
            for _ in range(22):
                nc.tensor.matmul(warm_ps[:], wl, wr, start=True, stop=True)
            warm_sink = work.tile([1, 1], F32, tag="warm_sink")
            nc.scalar.copy(out=warm_sink[:], in_=warm_ps[0:1, 0:1])

            # ---------- both pairs: weights broadcast + w_eff first ----------
            weffs = []
            for j in range(2):
                wfbc_ps = pspool.tile([128, NB], F32, tag="small")
                nc.tensor.matmul(
                    wfbc_ps[:], selsb[:, j, :], wf[:], start=True, stop=True
                )
                wfbc = work.tile([128, NB], F32, tag=f"wfbc{j}")
                nc.scalar.copy(out=wfbc[:], in_=wfbc_ps[:])

                weff = work.tile([128, NTAP, COUT], CONV_DT, tag=f"weff{j}")
                for lo, hi in ((0, 5), (5, NTAP)):
                    nc.vector.tensor_scalar_mul(
                        out=weff[:, lo:hi], in0=cwp2[:, 0, lo:hi], scalar1=wfbc[:, 0:1]
                    )
                    for n in range(1, NB):
                        nc.vector.scalar_tensor_tensor(
                            out=weff[:, lo:hi],
                            in0=cwp2[:, n, lo:hi],
                            scalar=wfbc[:, n : n + 1],
                            in1=weff[:, lo:hi],
                            op0=ALU.mult,
                            op1=ALU.add,
                        )
                weffs.append(weff)

            # ---------- PE warmup: keep HAM busy until conv starts ----------
            warm_ps = pspool.tile([128, 512], F32, tag="warm")
            wl = ident[:].bitcast(mybir.dt.bfloat16)[:, 0:128]
            wr = w1sb[:].rearrange("p c m -> p (c m)").bitcast(mybir.dt.bfloat16)[:, 0:512]
            for _ in range(22):
                nc.tensor.matmul(warm_ps[:], wl, wr, start=True, stop=True)
            warm_sink = work.tile([1, 1], F32, tag="warm_sink")
            nc.scalar.copy(out=warm_sink[:], in_=warm_ps[0:1, 0:1])

            # ---------- conv ----------
            for j in range(2):
                weff = weffs[j]
                for ch in range(NCHUNK):
                    h0 = ch * CHUNK_ROWS
                    psA = psconv.tile([COUT, NFREE], F32, tag="psA")
                    psB = psconv.tile([COUT, NFREE], F32, tag="psB")
                    for t in range(NTAP):
                        off = h0 * W + TAP_OFF[t]
                        nc.tensor.matmul(
                            psA[:],
                            weff[0:64, t, :],
                            xt[j][0:64, off : off + NFREE],
                            start=(t == 0),
                            stop=(t == NTAP - 1),
                            tile_position=(0, 0) if PAIRED else None,
                        )
                        nc.tensor.matmul(
                            psB[:],
                            weff[64:128, t, :],
                            xt[j][64:128, off : off + NFREE],
                            start=(t == 0),
                            stop=(t == NTAP - 1),
                            tile_position=(64, 0) if PAIRED else None,
                        )
                    for half, ps in ((0, psA), (1, psB)):
                        b = 2 * j + half
                        st = stage.tile([COUT, CHUNK_ROWS, OW], F32, tag="st")
                        psv = ps[:].rearrange("p (r w) -> p r w", w=W)[:, :, 0:OW]
                        nc.scalar.activation(
                            out=st[:],
                            in_=psv,
                            func=AF.Identity,
                            bias=beff[:, b : b + 1],
                            scale=1.0,
                        )
                        nc.sync.dma_start(
                            out=out[b, :, h0 : h0 + CHUNK_ROWS, :], in_=st[:]
                        )

    fix_sync_waits(nc)
    return nc


_NC = None


def _get_nc():
    global _NC
    if _NC is None:
        _NC = build()
    return _NC


def make_in_maps(inputs):
    x = np.ascontiguousarray(np.asarray(inputs["x"], dtype=np.float32))
    rvec = np.ascontiguousarray(np.asarray(inputs["routing_vector"], dtype=np.float32))
    W1 = np.ascontiguousarray(np.asarray(inputs["W1"], dtype=np.float32))
    b1 = np.ascontiguousarray(np.asarray(inputs["b1"], dtype=np.float32)).reshape(HID, 1)
    W2 = np.ascontiguousarray(np.asarray(inputs["W2"], dtype=np.float32))
    b2 = np.ascontiguousarray(np.asarray(inputs["b2"], dtype=np.float32)).reshape(EDIM, 1)
    emb = np.ascontiguousarray(np.asarray(inputs["emb"], dtype=np.float32))
    conv_w = np.asarray(inputs["conv_w"], dtype=np.float32)
    conv_b = np.ascontiguousarray(np.asarray(inputs["conv_b"], dtype=np.float32))
    # conv_w[n, co, ci, ky, kx] -> cwp[ci, n, (ky kx), co]
    cwpa = np.ascontiguousarray(
        conv_w.transpose(2, 0, 3, 4, 1).reshape(CIN, NB, NTAP, COUT)
    )
    selm = np.zeros((2, BLOC, 128), np.float32)
    for j in range(2):
        selm[j, 2 * j, 0:64] = 1.0
        selm[j, 2 * j + 1, 64:128] = 1.0
    identm = np.eye(128, dtype=np.float32)
    in_maps = []
    for c in range(NCORES):
        in_maps.append(
            {
                "x": np.ascontiguousarray(x[BLOC * c : BLOC * (c + 1)]),
                "rv": np.ascontiguousarray(rvec[BLOC * c : BLOC * (c + 1)]),
                "w1": W1,
                "bias1": b1,
                "w2": W2,
                "bias2": b2,
                "emb": emb,
                "cwp": cwpa,
                "cb": conv_b,
                "sel": selm,
                "identin": identm,
            }
        )
    return in_maps


def kernel(**inputs):
    from concourse.bass_utils import run_bass_kernel_spmd

    nc = _get_nc()
    in_maps = make_in_maps(inputs)
    res = run_bass_kernel_spmd(nc, in_maps, core_ids=list(range(NCORES)))
    return np.concatenate([r["out"] for r in res.results], axis=0)



# revision 3
# speedup vs baseline: 1.4616x; 1.4616x over previous
"""MoE routing layer on 8 Trainium2 NeuronCores (data-parallel over batch).

Per core (4 samples):
  routing MLP -> exp(cosine sim vs embeddings) -> unnormalized weights
  e[4,10]; w_eff[b] = sum_n e[b,n] * conv_w[n] on DVE (conv linear in
  weights); conv = 9 shifted bf16 matmuls over the flat 58-wide grid,
  all 4 samples concurrent via 4-quadrant PE tiling (64x64 tiles at
  (0,0),(64,64),(64,0),(0,64)); softmax normalization (1/sum e) and
  conv bias are folded into the PSUM->SBUF staging op on ACT.

Everything heavy is bf16 (x, conv weights, w_eff, output staging);
psum accumulation is fp32. Host pre-packs: x into the pair layout,
rv transposed, emb normalized, conv weights [ci-dup128, n, tap, co].
"""
import sys

sys.path.insert(0, "/opt/trn_rl_repo")

import ml_dtypes
import numpy as np

import concourse.bass as bass
import concourse.mybir as mybir
from concourse.tile import TileContext

F32 = mybir.dt.float32
BF16 = mybir.dt.bfloat16
AF = mybir.ActivationFunctionType
ALU = mybir.AluOpType
AX = mybir.AxisListType

NCORES = 8
BLOC = 4           # samples per core
CIN = 64
COUT = 64
H = W = 58
HW = H * W         # 3364
HWP = HW + 4       # padded (last-chunk tap overrun, host-zeroed)
OH = OW = 56
NB = 10            # experts
EDIM = 64
RSIZE = 512
HID = 128
NTAP = 9
CHUNK_ROWS = 8
NCHUNK = 7         # 7*8 = 56 output rows
NFREE = CHUNK_ROWS * W  # 464 <= 512 (one PSUM bank)
TAP_OFF = [dy * W + dx for dy in range(3) for dx in range(3)]
# x DMA pieces by input-row range (chunk ch needs rows 8ch..8ch+9)
XPIECES = [(0, 24), (24, 42), (42, 58)]
# expert order follows cwp DMA arrival (ring B gets 4:10 first)
EXPERT_ORDER = [4, 5, 6, 7, 8, 9, 0, 1, 2, 3]
CWP_RING_B = [(4, 7), (7, 10)]   # on scalar-engine HWDGE ring
CWP_RING_S = [(0, 2), (2, 4)]    # on sync-engine ring, after smalls


def fix_sync_waits(nc, cap=2):
    """This walrus build allows at most `cap` sem waits per instruction.
    Splice same-engine NoOps carrying the excess waits right before any
    over-subscribed instruction (waits happen earlier => same semantics)."""
    uid = [0]
    for f in nc.m.functions:
        for blk in f.blocks:
            insts = blk.instructions  # live list
            i = 0
            while i < len(insts):
                inst = insts[i]
                si = inst.sync_info
                waits = list(si.on_wait) if si and si.on_wait else []
                icap = 1
                if len(waits) <= icap:
                    i += 1
                    continue
                keep, excess = waits[-icap:], waits[:-icap]
                for k in range(0, len(excess), icap):
                    nop = mybir.InstNoOp(
                        name=f"{inst.name}-wsplit{uid[0]}", ins=[], outs=[]
                    )
                    uid[0] += 1
                    nop.engine = inst.engine
                    nop.sync_info = mybir.SyncInfo(
                        on_wait=excess[k : k + icap], on_update=[]
                    )
                    nc.register_instruction(nop, overwrite=True)
                    insts.insert(i, nop)
                    i += 1
                inst.sync_info = mybir.SyncInfo(
                    on_wait=keep,
                    on_update=list(si.on_update) if si and si.on_update else [],
                )
                i += 1
    return nc


def build():
    nc = bass.Bass()
    # partition layout p (all [128,...] tensors): p<64 -> ci=p, samples
    # {j0: b0, j1: b3}; p>=64 -> ci=p-64, samples {j0: b1, j1: b2}
    xall = nc.dram_tensor("xall", [128, 2, HWP], BF16, kind="ExternalInput")
    cwpd = nc.dram_tensor("cwpd", [128, NB, NTAP, COUT], BF16, kind="ExternalInput")
    rvtd = nc.dram_tensor("rvtd", [128, 4, BLOC], BF16, kind="ExternalInput")
    w1d = nc.dram_tensor("w1d", [128, 4, HID], BF16, kind="ExternalInput")
    w2d = nc.dram_tensor("w2d", [HID, EDIM], F32, kind="ExternalInput")
    b1d = nc.dram_tensor("b1d", [HID, 1], F32, kind="ExternalInput")
    b2d = nc.dram_tensor("b2d", [EDIM, 1], F32, kind="ExternalInput")
    embntd = nc.dram_tensor("embntd", [EDIM, NB], F32, kind="ExternalInput")
    cb2d = nc.dram_tensor("cb2d", [128, NB], F32, kind="ExternalInput")
    seld = nc.dram_tensor("seld", [BLOC, 4, 128], F32, kind="ExternalInput")
    out = nc.dram_tensor("out", [BLOC, COUT, OH, OW], BF16, kind="ExternalOutput")

    with TileContext(nc) as tc:
        with (
            tc.tile_pool(name="consts", bufs=1) as consts,
            tc.tile_pool(name="work", bufs=2) as work,
            tc.tile_pool(name="stage", bufs=2) as stage,
            tc.tile_pool(name="ps", bufs=2, space="PSUM") as pspool,
            tc.tile_pool(name="psconv", bufs=2, space="PSUM") as psconv,
        ):
            # ---------- input DMAs; order = issue priority ----------
            # sync ring: routing smalls, then cwp experts 0:4, then x
            rvt = consts.tile([128, 4, BLOC], BF16, tag="rvt")
            nc.sync.dma_start(out=rvt[:], in_=rvtd[:])
            w1sb = consts.tile([128, 4, HID], BF16, tag="w1sb")
            nc.sync.dma_start(out=w1sb[:], in_=w1d[:])
            w2sb = consts.tile([HID, EDIM], F32, tag="w2sb")
            nc.sync.dma_start(out=w2sb[:], in_=w2d[:])
            b1sb = consts.tile([HID, 1], F32, tag="b1sb")
            nc.sync.dma_start(out=b1sb[:], in_=b1d[:])
            b2sb = consts.tile([EDIM, 1], F32, tag="b2sb")
            nc.sync.dma_start(out=b2sb[:], in_=b2d[:])
            embnt = consts.tile([EDIM, NB], F32, tag="embnt")
            nc.sync.dma_start(out=embnt[:], in_=embntd[:])
            cb2 = consts.tile([128, NB], F32, tag="cb2")
            nc.sync.dma_start(out=cb2[:], in_=cb2d[:])
            selsb = consts.tile([BLOC, 4, 128], F32, tag="selsb")
            nc.sync.dma_start(out=selsb[:], in_=seld[:])
            ones64 = consts.tile([EDIM, 1], F32, tag="ones64")
            nc.vector.memset(ones64[:], 1.0)

            cwp2 = consts.tile([128, NB, NTAP, COUT], BF16, tag="cwp2")
            for lo, hi in CWP_RING_B:
                nc.scalar.dma_start(out=cwp2[:, lo:hi], in_=cwpd[:, lo:hi])
            for lo, hi in CWP_RING_S:
                nc.sync.dma_start(out=cwp2[:, lo:hi], in_=cwpd[:, lo:hi])

            xsb = consts.tile([128, 2, HWP], BF16, tag="xsb")
            for ra, rb in XPIECES:
                a = ra * W
                b = rb * W if rb < H else HWP
                nc.sync.dma_start(out=xsb[:, :, a:b], in_=xall[:, :, a:b])

            # ---------- routing MLP (f32 except the big W1 matmul) ----------
            h1 = pspool.tile([HID, BLOC], F32, tag="small")
            for c in range(4):
                nc.tensor.matmul(
                    h1[:], w1sb[:, c, :], rvt[:, c, :], start=(c == 0), stop=(c == 3)
                )
            h1r = work.tile([HID, BLOC], F32, tag="h1r")
            nc.scalar.activation(
                out=h1r[:], in_=h1[:], func=AF.Relu, bias=b1sb[:], scale=1.0
            )
            rps = pspool.tile([EDIM, BLOC], F32, tag="small")
            nc.tensor.matmul(rps[:], w2sb[:], h1r[:], start=True, stop=True)
            rsb = work.tile([EDIM, BLOC], F32, tag="rsb")
            nc.scalar.activation(
                out=rsb[:], in_=rps[:], func=AF.Identity, bias=b2sb[:], scale=1.0
            )

            # 1/||r|| (emb pre-normalized on host)
            rsq = work.tile([EDIM, BLOC], F32, tag="rsq")
            nc.vector.tensor_mul(rsq[:], rsb[:], rsb[:])
            nsq = pspool.tile([BLOC, 1], F32, tag="small")
            nc.tensor.matmul(nsq[:], rsq[:], ones64[:], start=True, stop=True)
            rln = work.tile([BLOC, 1], F32, tag="rln")
            nc.scalar.activation(out=rln[:], in_=nsq[:], func=AF.Ln)
            rinv = work.tile([BLOC, 1], F32, tag="rinv")
            nc.scalar.activation(out=rinv[:], in_=rln[:], func=AF.Exp, scale=-0.5)

            # e = exp(cos) directly: cos in [-1,1] so no max-subtraction
            simps = pspool.tile([BLOC, NB], F32, tag="small")
            nc.tensor.matmul(simps[:], rsb[:], embnt[:], start=True, stop=True)
            ex = work.tile([BLOC, NB], F32, tag="ex")
            nc.scalar.activation(out=ex[:], in_=simps[:], func=AF.Exp, scale=rinv[:])

            # broadcast e-weights to partition layouts: j0=(b0|b2), j1=(b3|b1)
            # for w_eff; j2=(b0|b1), j3=(b2|b3) for the per-bank bias
            wfbc = []
            for j in range(4):
                ps = pspool.tile([128, NB], F32, tag="small")
                nc.tensor.matmul(ps[:], selsb[:, j, :], ex[:], start=True, stop=True)
                t = work.tile([128, NB], F32, tag=f"wfbc{j}")
                nc.scalar.copy(out=t[:], in_=ps[:])
                wfbc.append(t)

            # softmax denom (does not block w_eff): sinv = 1/sum(e)
            s = work.tile([BLOC, 1], F32, tag="s")
            nc.vector.tensor_reduce(s[:], ex[:], axis=AX.X, op=ALU.add)
            sinv = work.tile([BLOC, 1], F32, tag="sinv")
            nc.vector.reciprocal(sinv[:], s[:])
            sinvbc = []
            for j in (2, 3):
                ps = pspool.tile([128, 1], F32, tag="small")
                nc.tensor.matmul(ps[:], selsb[:, j, :], sinv[:], start=True, stop=True)
                t = work.tile([128, 1], F32, tag=f"sinvbc{j}")
                nc.scalar.copy(out=t[:], in_=ps[:])
                sinvbc.append(t)

            # per-bank bias (e-weighted, pre-scaled by sinv):
            # beff[p] = sinv_bc[p] * sum_n wfbc_bias[p,n]*cb2[p,n]
            beff = []
            for k, j in enumerate((2, 3)):
                junk = work.tile([128, NB], F32, tag="bjunk")
                acc = work.tile([128, 1], F32, tag=f"bacc{j}")
                nc.vector.scalar_tensor_tensor(
                    out=junk[:], in0=wfbc[j], scalar=1.0, in1=cb2[:],
                    op0=ALU.mult, op1=ALU.mult, accum_out=acc[:],
                )
                t = work.tile([128, 1], F32, tag=f"beff{j}")
                nc.vector.tensor_mul(t[:], acc[:], sinvbc[k][:])
                beff.append(t)

            # ---------- w_eff on DVE (bf16, two chains over col-halves) ----------
            # weff[p, c, t, m]: c=0 -> (b0|b2) weights, c=1 -> (b3|b1)
            weff = consts.tile([128, 2, NTAP, COUT], BF16, tag="weff")
            for c in range(2):
                for k, n in enumerate(EXPERT_ORDER):
                    if k == 0:
                        nc.vector.tensor_scalar_mul(
                            out=weff[:, c], in0=cwp2[:, n],
                            scalar1=wfbc[c][:, n : n + 1],
                        )
                    else:
                        nc.vector.scalar_tensor_tensor(
                            out=weff[:, c], in0=cwp2[:, n],
                            scalar=wfbc[c][:, n : n + 1], in1=weff[:, c],
                            op0=ALU.mult, op1=ALU.add,
                        )

            # ---------- conv: 7 chunks x 9 taps x 4 quadrant MMs ----------
            # quadrants: b0=(0,0) psA-low, b1=(64,64) psA-high,
            #            b2=(64,0) psB-low, b3=(0,64) psB-high
            stages = []  # (bank, group_h0, tile, rows_filled)
            gtile = {}
            for ch in range(NCHUNK):
                h0 = ch * CHUNK_ROWS
                base = h0 * W
                psA = psconv.tile([128, NFREE], F32, tag="A")
                psB = psconv.tile([128, NFREE], F32, tag="B")
                for t in range(NTAP):
                    off = base + TAP_OFF[t]
                    st_, sp = (t == 0), (t == NTAP - 1)
                    nc.tensor.matmul(
                        psA[0:64, :], weff[0:64, 0, t, :],
                        xsb[0:64, 0, off : off + NFREE], start=st_, stop=sp,
                    )
                    nc.tensor.matmul(
                        psA[64:128, :], weff[64:128, 1, t, :],
                        xsb[64:128, 0, off : off + NFREE], start=st_, stop=sp,
                    )
                    nc.tensor.matmul(
                        psB[0:64, :], weff[64:128, 0, t, :],
                        xsb[64:128, 1, off : off + NFREE], start=st_, stop=sp,
                    )
                    nc.tensor.matmul(
                        psB[64:128, :], weff[0:64, 1, t, :],
                        xsb[0:64, 1, off : off + NFREE], start=st_, stop=sp,
                    )
                # stage: out = psum * sinv + bias, f32->bf16, trim to 56 cols
                g = ch // 2
                r = (ch % 2) * CHUNK_ROWS
                for bi, ps in ((0, psA), (1, psB)):
                    key = (bi, g)
                    if key not in gtile:
                        rows = CHUNK_ROWS if ch == NCHUNK - 1 else 2 * CHUNK_ROWS
                        stile = stage.tile(
                            [128, rows, OW], BF16, tag=f"st{bi}", name=f"st{bi}g{g}"
                        )
                        gtile[key] = (stile, g * 2 * CHUNK_ROWS, rows)
                    stile, gh0, rows = gtile[key]
                    psv = ps[:].rearrange("p (r w) -> p r w", w=W)[:, :, 0:OW]
                    nc.scalar.activation(
                        out=stile[:, r : r + CHUNK_ROWS, :], in_=psv,
                        func=AF.Identity, bias=beff[bi][:], scale=sinvbc[bi][:],
                    )
                    if r + CHUNK_ROWS >= rows:  # group complete -> DMA out
                        for half in range(2):
                            b = 2 * bi + half
                            nc.scalar.dma_start(
                                out=out[b, :, gh0 : gh0 + rows, :],
                                in_=stile[64 * half : 64 * half + 64],
                            )

    fix_sync_waits(nc)
    return nc


_NC = None


def _get_nc():
    global _NC
    if _NC is None:
        _NC = build()
    return _NC


def make_in_maps(inputs):
    bf16 = ml_dtypes.bfloat16
    x = np.asarray(inputs["x"], dtype=np.float32)            # [32,64,58,58]
    rvec = np.asarray(inputs["routing_vector"], dtype=np.float32)
    W1 = np.asarray(inputs["W1"], dtype=np.float32)
    b1 = np.asarray(inputs["b1"], dtype=np.float32).reshape(HID, 1)
    b2 = np.asarray(inputs["b2"], dtype=np.float32).reshape(EDIM, 1)
    emb = np.asarray(inputs["emb"], dtype=np.float32)
    conv_w = np.asarray(inputs["conv_w"], dtype=np.float32)
    conv_b = np.asarray(inputs["conv_b"], dtype=np.float32)

    # emb normalized on host; transposed to [EDIM, NB]
    embn = emb / (np.linalg.norm(emb, axis=-1, keepdims=True) + 1e-8)
    embnt = np.ascontiguousarray(embn.T)

    # conv_w[n, co, ci, ky, kx] -> [ci(dup 128), n, tap, co] bf16
    cwp = conv_w.transpose(2, 0, 3, 4, 1).reshape(CIN, NB, NTAP, COUT)
    cwpd = np.ascontiguousarray(
        np.concatenate([cwp, cwp], axis=0).astype(bf16)
    )

    # W1 [512,128] -> [128, 4, 128] bf16 (k-chunked for 4 accum matmuls)
    w1p = np.ascontiguousarray(
        W1.reshape(4, 128, HID).transpose(1, 0, 2).astype(bf16)
    )

    # conv bias dup'd: cb2[p, n] = conv_b[n, p%64]
    cb2 = np.ascontiguousarray(np.tile(conv_b.T, (2, 1)))    # [128, 10]

    # broadcast selectors: j0=(b0|b2), j1=(b3|b1), j2=(b0|b1), j3=(b2|b3)
    pairs = [(0, 2), (3, 1), (0, 1), (2, 3)]
    selm = np.zeros((BLOC, 4, 128), np.float32)
    for j, (blo, bhi) in enumerate(pairs):
        selm[blo, j, 0:64] = 1.0
        selm[bhi, j, 64:128] = 1.0

    in_maps = []
    for c in range(NCORES):
        xs = x[BLOC * c : BLOC * (c + 1)].reshape(BLOC, CIN, HW)
        # xall[p, j, i]: p<64: (j0: b0, j1: b3); p>=64: (j0: b1, j1: b2)
        xa = np.zeros((128, 2, HWP), np.float32)
        xa[0:64, 0, 0:HW] = xs[0]
        xa[64:128, 0, 0:HW] = xs[1]
        xa[64:128, 1, 0:HW] = xs[2]
        xa[0:64, 1, 0:HW] = xs[3]
        rvs = rvec[BLOC * c : BLOC * (c + 1)]                # [4, 512]
        rvt = np.ascontiguousarray(
            rvs.T.reshape(4, 128, BLOC).transpose(1, 0, 2).astype(bf16)
        )
        in_maps.append(
            {
                "xall": np.ascontiguousarray(xa.astype(bf16)),
                "cwpd": cwpd,
                "rvtd": rvt,
                "w1d": w1p,
                "w2d": np.ascontiguousarray(np.asarray(inputs["W2"], np.float32)),
                "b1d": b1,
                "b2d": b2,
                "embntd": embnt,
                "cb2d": cb2,
                "seld": selm,
            }
        )
    return in_maps


def kernel(**inputs):
    from concourse.bass_utils import run_bass_kernel_spmd

    nc = _get_nc()
    in_maps = make_in_maps(inputs)
    res = run_bass_kernel_spmd(nc, in_maps, core_ids=list(range(NCORES)))
    return np.concatenate(
        [np.asarray(r["out"]).astype(np.float32) for r in res.results], axis=0
    )


# revision 13
# speedup vs baseline: 1.5694x; 1.0738x over previous
"""MoE routing layer on 8 Trainium2 NeuronCores (data-parallel over batch).

Per core (4 samples):
  routing MLP -> exp(cosine sim vs embeddings) -> unnormalized weights
  e[4,10]; w_eff[b] = sum_n e[b,n] * conv_w[n] on DVE (conv linear in
  weights); conv = 9 shifted bf16 matmuls over the flat 58-wide grid,
  all 4 samples concurrent via 4-quadrant PE tiling (64x64 tiles at
  (0,0),(64,64),(64,0),(0,64)); softmax normalization (1/sum e) and
  conv bias are folded into the PSUM->SBUF staging op on ACT.

Everything heavy is bf16 (x, conv weights, w_eff, output staging);
psum accumulation is fp32. Host pre-packs: x into the pair layout,
rv transposed, emb normalized, conv weights [ci-dup128, n, tap, co].
"""
import sys

sys.path.insert(0, "/opt/trn_rl_repo")

import ml_dtypes
import numpy as np

import concourse.bass as bass
import concourse.mybir as mybir
from concourse.tile import TileContext

F32 = mybir.dt.float32
BF16 = mybir.dt.bfloat16
AF = mybir.ActivationFunctionType
ALU = mybir.AluOpType
AX = mybir.AxisListType

NCORES = 8
BLOC = 4           # samples per core
CIN = 64
COUT = 64
H = W = 58
HW = H * W         # 3364
HWP = HW + 4       # padded (last-chunk tap overrun, host-zeroed)
OH = OW = 56
NB = 10            # experts
EDIM = 64
RSIZE = 512
HID = 128
NTAP = 9
CHUNK_ROWS = 8
NCHUNK = 7         # 7*8 = 56 output rows
NFREE = CHUNK_ROWS * W  # 464 <= 512 (one PSUM bank)
TAP_OFF = [dy * W + dx for dy in range(3) for dx in range(3)]
# x DMA pieces by input-row range, overlapping so chunks 0-3 read piece 0
# and chunks 4-6 read piece 1 (big contiguous DMA descriptors per piece)
XP0_ROWS = (0, 34)    # chunks 0-3 (input rows 0..33)
XP1_ROWS = (26, 58)   # chunks 4-6 (input rows 32..57), +4 pad cols
XP0_N = 34 * W + 4    # 1976 (last-tap overrun of chunk 3)
XP1_N = 32 * W + 4    # 1860
CWP_GROUPS = [(0, 5), (5, 10)]
# out stage groups: chunk ranges sharing one staging tile / output DMA
OGROUPS = [(0, 2), (2, 4), (4, 7)]


def fix_sync_waits(nc, cap=2):
    """This walrus build allows at most `cap` sem waits per instruction.
    Splice same-engine NoOps carrying the excess waits right before any
    over-subscribed instruction (waits happen earlier => same semantics)."""
    uid = [0]
    for f in nc.m.functions:
        for blk in f.blocks:
            insts = blk.instructions  # live list
            i = 0
            while i < len(insts):
                inst = insts[i]
                si = inst.sync_info
                waits = list(si.on_wait) if si and si.on_wait else []
                icap = 1
                if len(waits) <= icap:
                    i += 1
                    continue
                keep, excess = waits[-icap:], waits[:-icap]
                for k in range(0, len(excess), icap):
                    nop = mybir.InstNoOp(
                        name=f"{inst.name}-wsplit{uid[0]}", ins=[], outs=[]
                    )
                    uid[0] += 1
                    nop.engine = inst.engine
                    nop.sync_info = mybir.SyncInfo(
                        on_wait=excess[k : k + icap], on_update=[]
                    )
                    nc.register_instruction(nop, overwrite=True)
                    insts.insert(i, nop)
                    i += 1
                inst.sync_info = mybir.SyncInfo(
                    on_wait=keep,
                    on_update=list(si.on_update) if si and si.on_update else [],
                )
                i += 1
    return nc


def build():
    nc = bass.Bass()
    # partition layout p (all [128,...] tensors): p<64 -> ci=p, samples
    # {j0: b0, j1: b3}; p>=64 -> ci=p-64, samples {j0: b1, j1: b2}
    xp0d = nc.dram_tensor("xp0d", [128, 2, XP0_N], BF16, kind="ExternalInput")
    xp1d = nc.dram_tensor("xp1d", [128, 2, XP1_N], BF16, kind="ExternalInput")
    cwpd = nc.dram_tensor("cwpd", [128, NB, NTAP, COUT], BF16, kind="ExternalInput")
    rvtd = nc.dram_tensor("rvtd", [128, 4, BLOC], BF16, kind="ExternalInput")
    w1d = nc.dram_tensor("w1d", [128, 4, HID], BF16, kind="ExternalInput")
    w2d = nc.dram_tensor("w2d", [HID, EDIM], F32, kind="ExternalInput")
    b1d = nc.dram_tensor("b1d", [HID, 1], F32, kind="ExternalInput")
    b2d = nc.dram_tensor("b2d", [EDIM, 1], F32, kind="ExternalInput")
    embntd = nc.dram_tensor("embntd", [EDIM, NB], F32, kind="ExternalInput")
    cb2d = nc.dram_tensor("cb2d", [128, NB], F32, kind="ExternalInput")
    seld = nc.dram_tensor("seld", [BLOC, 4, 128], F32, kind="ExternalInput")
    out = nc.dram_tensor("out", [BLOC, COUT, OH, OW], BF16, kind="ExternalOutput")

    with TileContext(nc) as tc:
        with (
            tc.tile_pool(name="consts", bufs=1) as consts,
            tc.tile_pool(name="work", bufs=2) as work,
            tc.tile_pool(name="stage", bufs=2) as stage,
            tc.tile_pool(name="ps", bufs=2, space="PSUM") as pspool,
            tc.tile_pool(name="psconv", bufs=2, space="PSUM") as psconv,
        ):
            # ---------- input DMAs; order = issue priority ----------
            # sync ring: routing smalls, then cwp experts 0:4, then x
            rvt = consts.tile([128, 4, BLOC], BF16, tag="rvt")
            nc.sync.dma_start(out=rvt[:], in_=rvtd[:])
            w1sb = consts.tile([128, 4, HID], BF16, tag="w1sb")
            nc.sync.dma_start(out=w1sb[:], in_=w1d[:])
            w2sb = consts.tile([HID, EDIM], F32, tag="w2sb")
            nc.sync.dma_start(out=w2sb[:], in_=w2d[:])
            b1sb = consts.tile([HID, 1], F32, tag="b1sb")
            nc.sync.dma_start(out=b1sb[:], in_=b1d[:])
            b2sb = consts.tile([EDIM, 1], F32, tag="b2sb")
            nc.sync.dma_start(out=b2sb[:], in_=b2d[:])
            embnt = consts.tile([EDIM, NB], F32, tag="embnt")
            nc.sync.dma_start(out=embnt[:], in_=embntd[:])
            cb2 = consts.tile([128, NB], F32, tag="cb2")
            nc.sync.dma_start(out=cb2[:], in_=cb2d[:])
            selsb = consts.tile([BLOC, 4, 128], F32, tag="selsb")
            nc.sync.dma_start(out=selsb[:], in_=seld[:])
            ones64 = consts.tile([EDIM, 1], F32, tag="ones64")
            nc.vector.memset(ones64[:], 1.0)

            cwp2 = consts.tile([128, NB, NTAP, COUT], BF16, tag="cwp2")
            for lo, hi in CWP_GROUPS:
                nc.sync.dma_start(out=cwp2[:, lo:hi], in_=cwpd[:, lo:hi])

            xs0 = consts.tile([128, 2, XP0_N], BF16, tag="xs0")
            nc.sync.dma_start(out=xs0[:], in_=xp0d[:])
            xs1 = consts.tile([128, 2, XP1_N], BF16, tag="xs1")
            nc.sync.dma_start(out=xs1[:], in_=xp1d[:])

            # ---------- routing MLP (f32 except the big W1 matmul) ----------
            h1 = pspool.tile([HID, BLOC], F32, tag="small")
            for c in range(4):
                nc.tensor.matmul(
                    h1[:], w1sb[:, c, :], rvt[:, c, :], start=(c == 0), stop=(c == 3)
                )
            h1r = work.tile([HID, BLOC], F32, tag="h1r")
            nc.scalar.activation(
                out=h1r[:], in_=h1[:], func=AF.Relu, bias=b1sb[:], scale=1.0
            )
            rps = pspool.tile([EDIM, BLOC], F32, tag="small")
            nc.tensor.matmul(rps[:], w2sb[:], h1r[:], start=True, stop=True)
            rsb = work.tile([EDIM, BLOC], F32, tag="rsb")
            nc.scalar.activation(
                out=rsb[:], in_=rps[:], func=AF.Identity, bias=b2sb[:], scale=1.0
            )

            # 1/||r|| (emb pre-normalized on host)
            rsq = work.tile([EDIM, BLOC], F32, tag="rsq")
            nc.vector.tensor_mul(rsq[:], rsb[:], rsb[:])
            nsq = pspool.tile([BLOC, 1], F32, tag="small")
            nc.tensor.matmul(nsq[:], rsq[:], ones64[:], start=True, stop=True)
            rln = work.tile([BLOC, 1], F32, tag="rln")
            nc.scalar.activation(out=rln[:], in_=nsq[:], func=AF.Ln)
            rinv = work.tile([BLOC, 1], F32, tag="rinv")
            nc.scalar.activation(out=rinv[:], in_=rln[:], func=AF.Exp, scale=-0.5)

            # e = exp(cos) directly: cos in [-1,1] so no max-subtraction
            simps = pspool.tile([BLOC, NB], F32, tag="small")
            nc.tensor.matmul(simps[:], rsb[:], embnt[:], start=True, stop=True)
            ex = work.tile([BLOC, NB], F32, tag="ex")
            nc.scalar.activation(out=ex[:], in_=simps[:], func=AF.Exp, scale=rinv[:])

            # broadcast e-weights to partition layouts: j0=(b0|b2), j1=(b3|b1)
            # for w_eff; j2=(b0|b1), j3=(b2|b3) for the per-bank bias
            wfbc = []
            for j in range(4):
                ps = pspool.tile([128, NB], F32, tag="small")
                nc.tensor.matmul(ps[:], selsb[:, j, :], ex[:], start=True, stop=True)
                t = work.tile([128, NB], F32, tag=f"wfbc{j}")
                nc.scalar.copy(out=t[:], in_=ps[:])
                wfbc.append(t)

            # softmax denom (does not block w_eff): sinv = 1/sum(e)
            s = work.tile([BLOC, 1], F32, tag="s")
            nc.vector.tensor_reduce(s[:], ex[:], axis=AX.X, op=ALU.add)
            sinv = work.tile([BLOC, 1], F32, tag="sinv")
            nc.vector.reciprocal(sinv[:], s[:])
            sinvbc = []
            for j in (2, 3):
                ps = pspool.tile([128, 1], F32, tag="small")
                nc.tensor.matmul(ps[:], selsb[:, j, :], sinv[:], start=True, stop=True)
                t = work.tile([128, 1], F32, tag=f"sinvbc{j}")
                nc.scalar.copy(out=t[:], in_=ps[:])
                sinvbc.append(t)

            # per-bank bias (e-weighted, pre-scaled by sinv):
            # beff[p] = sinv_bc[p] * sum_n wfbc_bias[p,n]*cb2[p,n]
            beff = []
            for k, j in enumerate((2, 3)):
                junk = work.tile([128, NB], F32, tag="bjunk")
                acc = work.tile([128, 1], F32, tag=f"bacc{j}")
                nc.vector.scalar_tensor_tensor(
                    out=junk[:], in0=wfbc[j], scalar=1.0, in1=cb2[:],
                    op0=ALU.mult, op1=ALU.mult, accum_out=acc[:],
                )
                t = work.tile([128, 1], F32, tag=f"beff{j}")
                nc.vector.tensor_mul(t[:], acc[:], sinvbc[k][:])
                beff.append(t)

            # ---------- w_eff (bf16): products on ACT+DVE, adds on DVE ----------
            # weff[p, c, t, m]: c=0 -> (b0|b2) weights, c=1 -> (b3|b1).
            # scalar_tensor_tensor measured 1x-rate (~870ns); tensor_scalar
            # products (2-4x) + tensor_tensor adds (2x) are cheaper, with
            # ~half the products offloaded to ACT copy-with-scale.
            weff = consts.tile([128, 2, NTAP, COUT], BF16, tag="weff")
            nc.vector.tensor_scalar_mul(
                out=weff[:, 0], in0=cwp2[:, 0], scalar1=wfbc[0][:, 0:1]
            )
            nc.scalar.activation(
                out=weff[:, 1], in_=cwp2[:, 0], func=AF.Copy,
                scale=wfbc[1][:, 0:1],
            )
            prods = [
                consts.tile([128, NTAP, COUT], BF16, tag=f"prod{c}{k}",
                            name=f"prod{c}{k}")
                for c in range(2) for k in range(2)
            ]
            for n in range(1, NB):
                for c in range(2):
                    p = prods[2 * c + (n % 2)]
                    if n in (2, 4):  # keep some products on DVE for balance
                        nc.vector.tensor_scalar_mul(
                            out=p[:], in0=cwp2[:, n], scalar1=wfbc[c][:, n : n + 1]
                        )
                    else:
                        nc.scalar.activation(
                            out=p[:], in_=cwp2[:, n], func=AF.Copy,
                            scale=wfbc[c][:, n : n + 1],
                        )
                    nc.vector.tensor_tensor(
                        out=weff[:, c], in0=weff[:, c], in1=p[:], op=ALU.add
                    )

            # ---------- PE warmup during w_eff (HAM un-throttle) ----------
            # 8 cold bf16 N=512 matmuls ~= 3.4us of PE busy; gated on the
            # last cwp group so they bridge the idle window before conv
            warm_ps = pspool.tile([128, 512], F32, tag="warm")
            for _ in range(8):
                nc.tensor.matmul(
                    warm_ps[:], w1sb[:, 0, :], cwp2[:, NB - 1, 0:8, :],
                    start=True, stop=True,
                )

            # ---------- conv: 7 chunks x 9 taps x 4 quadrant MMs ----------
            # quadrants: b0=(0,0) psA-low, b1=(64,64) psA-high,
            #            b2=(64,0) psB-low, b3=(0,64) psB-high
            gtile = {}
            g_of_chunk = {}
            for gi, (clo, chi) in enumerate(OGROUPS):
                for ch in range(clo, chi):
                    g_of_chunk[ch] = (gi, clo, chi)
            for ch in range(NCHUNK):
                h0 = ch * CHUNK_ROWS
                # chunks 0-3 read piece 0 (rows 0..33), 4-6 piece 1 (rows 26..)
                xs = xs0 if ch < 4 else xs1
                base = (h0 - (0 if ch < 4 else XP1_ROWS[0])) * W
                psA = psconv.tile([128, NFREE], F32, tag="A")
                psB = psconv.tile([128, NFREE], F32, tag="B")
                for t in range(NTAP):
                    off = base + TAP_OFF[t]
                    st_, sp = (t == 0), (t == NTAP - 1)
                    nc.tensor.matmul(
                        psA[0:64, :], weff[0:64, 0, t, :],
                        xs[0:64, 0, off : off + NFREE], start=st_, stop=sp,
                    )
                    nc.tensor.matmul(
                        psA[64:128, :], weff[64:128, 1, t, :],
                        xs[64:128, 0, off : off + NFREE], start=st_, stop=sp,
                    )
                    nc.tensor.matmul(
                        psB[0:64, :], weff[64:128, 0, t, :],
                        xs[64:128, 1, off : off + NFREE], start=st_, stop=sp,
                    )
                    nc.tensor.matmul(
                        psB[64:128, :], weff[0:64, 1, t, :],
                        xs[0:64, 1, off : off + NFREE], start=st_, stop=sp,
                    )
                # stage: out = psum * sinv + bias, f32->bf16, trim to 56 cols
                # bankA on ACT, bankB on DVE so neither engine paces the PE
                gi, clo, chi = g_of_chunk[ch]
                rows = (chi - clo) * CHUNK_ROWS
                r = (ch - clo) * CHUNK_ROWS
                for bi, ps in ((0, psA), (1, psB)):
                    key = (bi, gi)
                    if key not in gtile:
                        stile = stage.tile(
                            [128, rows, OW], BF16, tag=f"st{bi}_{rows}",
                            name=f"st{bi}g{gi}",
                        )
                        gtile[key] = stile
                    stile = gtile[key]
                    psv = ps[:].rearrange("p (r w) -> p r w", w=W)[:, :, 0:OW]
                    if bi == 0:
                        nc.scalar.activation(
                            out=stile[:, r : r + CHUNK_ROWS, :], in_=psv,
                            func=AF.Identity, bias=beff[bi][:], scale=sinvbc[bi][:],
                        )
                    else:
                        nc.vector.tensor_scalar(
                            out=stile[:, r : r + CHUNK_ROWS, :], in0=psv,
                            scalar1=sinvbc[bi][:], scalar2=beff[bi][:],
                            op0=ALU.mult, op1=ALU.add,
                        )
                    if ch == chi - 1:  # group complete -> DMA out (own queue)
                        gh0 = clo * CHUNK_ROWS
                        for half in range(2):
                            b = 2 * bi + half
                            nc.gpsimd.dma_start(
                                out=out[b, :, gh0 : gh0 + rows, :],
                                in_=stile[64 * half : 64 * half + 64],
                            )

    fix_sync_waits(nc)
    return nc


_NC = None


def _get_nc():
    global _NC
    if _NC is None:
        _NC = build()
    return _NC


def make_in_maps(inputs):
    bf16 = ml_dtypes.bfloat16
    x = np.asarray(inputs["x"], dtype=np.float32)            # [32,64,58,58]
    rvec = np.asarray(inputs["routing_vector"], dtype=np.float32)
    W1 = np.asarray(inputs["W1"], dtype=np.float32)
    b1 = np.asarray(inputs["b1"], dtype=np.float32).reshape(HID, 1)
    b2 = np.asarray(inputs["b2"], dtype=np.float32).reshape(EDIM, 1)
    emb = np.asarray(inputs["emb"], dtype=np.float32)
    conv_w = np.asarray(inputs["conv_w"], dtype=np.float32)
    conv_b = np.asarray(inputs["conv_b"], dtype=np.float32)

    # emb normalized on host; transposed to [EDIM, NB]
    embn = emb / (np.linalg.norm(emb, axis=-1, keepdims=True) + 1e-8)
    embnt = np.ascontiguousarray(embn.T)

    # conv_w[n, co, ci, ky, kx] -> [ci(dup 128), n, tap, co] bf16
    cwp = conv_w.transpose(2, 0, 3, 4, 1).reshape(CIN, NB, NTAP, COUT)
    cwpd = np.ascontiguousarray(
        np.concatenate([cwp, cwp], axis=0).astype(bf16)
    )

    # W1 [512,128] -> [128, 4, 128] bf16 (k-chunked for 4 accum matmuls)
    w1p = np.ascontiguousarray(
        W1.reshape(4, 128, HID).transpose(1, 0, 2).astype(bf16)
    )

    # conv bias dup'd: cb2[p, n] = conv_b[n, p%64]
    cb2 = np.ascontiguousarray(np.tile(conv_b.T, (2, 1)))    # [128, 10]

    # broadcast selectors: j0=(b0|b2), j1=(b3|b1), j2=(b0|b1), j3=(b2|b3)
    pairs = [(0, 2), (3, 1), (0, 1), (2, 3)]
    selm = np.zeros((BLOC, 4, 128), np.float32)
    for j, (blo, bhi) in enumerate(pairs):
        selm[blo, j, 0:64] = 1.0
        selm[bhi, j, 64:128] = 1.0

    in_maps = []
    for c in range(NCORES):
        xs = x[BLOC * c : BLOC * (c + 1)].reshape(BLOC, CIN, HW)
        # x[p, j, i]: p<64: (j0: b0, j1: b3); p>=64: (j0: b1, j1: b2)
        xa = np.zeros((128, 2, HWP), np.float32)
        xa[0:64, 0, 0:HW] = xs[0]
        xa[64:128, 0, 0:HW] = xs[1]
        xa[64:128, 1, 0:HW] = xs[2]
        xa[0:64, 1, 0:HW] = xs[3]
        xp0 = np.ascontiguousarray(
            xa[:, :, XP0_ROWS[0] * W : XP0_ROWS[0] * W + XP0_N].astype(bf16)
        )
        xp1 = np.ascontiguousarray(
            xa[:, :, XP1_ROWS[0] * W : XP1_ROWS[0] * W + XP1_N].astype(bf16)
        )
        rvs = rvec[BLOC * c : BLOC * (c + 1)]                # [4, 512]
        rvt = np.ascontiguousarray(
            rvs.T.reshape(4, 128, BLOC).transpose(1, 0, 2).astype(bf16)
        )
        in_maps.append(
            {
                "xp0d": xp0,
                "xp1d": xp1,
                "cwpd": cwpd,
                "rvtd": rvt,
                "w1d": w1p,
                "w2d": np.ascontiguousarray(np.asarray(inputs["W2"], np.float32)),
                "b1d": b1,
                "b2d": b2,
                "embntd": embnt,
                "cb2d": cb2,
                "seld": selm,
            }
        )
    return in_maps


def kernel(**inputs):
    from concourse.bass_utils import run_bass_kernel_spmd

    nc = _get_nc()
    in_maps = make_in_maps(inputs)
    res = run_bass_kernel_spmd(nc, in_maps, core_ids=list(range(NCORES)))
    return np.concatenate(
        [np.asarray(r["out"]).astype(np.float32) for r in res.results], axis=0
    )


# revision 17
# speedup vs baseline: 1.8547x; 1.1818x over previous
"""MoE routing layer on 8 Trainium2 NeuronCores (data-parallel over batch).

Per core (4 samples):
  routing MLP -> exp(cosine sim vs embeddings) -> unnormalized weights
  e[4,10]; w_eff[b] = sum_n e[b,n] * conv_w[n] (conv linear in weights);
  conv = 9 shifted bf16 matmuls over the flat 58-wide grid, all 4
  samples concurrent via 4-quadrant PE tiling; softmax normalization
  (1/sum e) and conv bias fold into the PSUM->SBUF staging op.

w_eff is built two ways concurrently: col-half 0 on the PE as a chain
of diagonal-stationary matmuls accumulating in PSUM (diag(wfbc_n) @
cwp_n), col-half 1 on DVE as tensor_scalar products + tensor_tensor
adds (ACT helps with late products). All heavy data is bf16; PSUM is
fp32. Host pre-packs: x in 4 row-band pieces (big DMA descriptors),
small tensors in one blob DMA, rv transposed, emb normalized,
conv weights as [ci-dup128, expert, tap, cout].
"""
import sys

sys.path.insert(0, "/opt/trn_rl_repo")

import ml_dtypes
import numpy as np

import concourse.bass as bass
import concourse.mybir as mybir
from concourse.tile import TileContext

F32 = mybir.dt.float32
BF16 = mybir.dt.bfloat16
AF = mybir.ActivationFunctionType
ALU = mybir.AluOpType
AX = mybir.AxisListType

NCORES = 8
BLOC = 4           # samples per core
CIN = 64
COUT = 64
H = W = 58
HW = H * W         # 3364
HWP = HW + 4
OH = OW = 56
NB = 10            # experts
EDIM = 64
RSIZE = 512
HID = 128
NTAP = 9
CHUNK_ROWS = 8
NCHUNK = 7
NFREE = CHUNK_ROWS * W  # 464 <= 512 (one PSUM bank)
TAP_OFF = [dy * W + dx for dy in range(3) for dx in range(3)]
# x pieces: (first input row, n rows). chunk ch (input rows 8ch..8ch+9)
# reads piece ch//2; pieces overlap by 2 rows; each padded +4 elems
XPIECES = [(0, 18), (16, 18), (32, 18), (48, 10)]
XPN = [r * W + 4 for _, r in XPIECES]
CWP_GROUPS = [(0, 5), (5, 10)]
OGROUPS = [(0, 2), (2, 4), (4, 6), (6, 7)]
# blob layout in fp32 columns: name -> (start, cols)
BL_RVT = (0, 8)        # bf16 [128, 4, 4]
BL_W1 = (8, 264)       # bf16 [128, 4, 128]
BL_W2 = (264, 328)     # f32 [128, 64]
BL_B1 = (328, 329)     # f32 [128, 1]
BL_B2 = (329, 330)     # f32 [64, 1]
BL_EMB = (330, 340)    # f32 [64, 10]
BL_CB = (340, 350)     # f32 [128, 10]
BL_ID = (350, 414)     # bf16 [128, 128] identity
BL_SEL = (414, 926)    # f32 [4, 4, 128] on partitions 0-3
NBLOB = 926


def fix_sync_waits(nc, cap=2):
    """This walrus build allows at most `cap` sem waits per instruction.
    Splice same-engine NoOps carrying the excess waits right before any
    over-subscribed instruction (waits happen earlier => same semantics)."""
    uid = [0]
    for f in nc.m.functions:
        for blk in f.blocks:
            insts = blk.instructions  # live list
            i = 0
            while i < len(insts):
                inst = insts[i]
                si = inst.sync_info
                waits = list(si.on_wait) if si and si.on_wait else []
                icap = 1
                if len(waits) <= icap:
                    i += 1
                    continue
                keep, excess = waits[-icap:], waits[:-icap]
                for k in range(0, len(excess), icap):
                    nop = mybir.InstNoOp(
                        name=f"{inst.name}-wsplit{uid[0]}", ins=[], outs=[]
                    )
                    uid[0] += 1
                    nop.engine = inst.engine
                    nop.sync_info = mybir.SyncInfo(
                        on_wait=excess[k : k + icap], on_update=[]
                    )
                    nc.register_instruction(nop, overwrite=True)
                    insts.insert(i, nop)
                    i += 1
                inst.sync_info = mybir.SyncInfo(
                    on_wait=keep,
                    on_update=list(si.on_update) if si and si.on_update else [],
                )
                i += 1
    return nc


def build():
    nc = bass.Bass()
    # partition layout p (all [128,...] tensors): p<64 -> ci=p, samples
    # {j0: b0, j1: b3}; p>=64 -> ci=p-64, samples {j0: b1, j1: b2}
    blobd = nc.dram_tensor("blobd", [128, NBLOB], F32, kind="ExternalInput")
    cwpd = nc.dram_tensor("cwpd", [128, NB, NTAP, COUT], BF16, kind="ExternalInput")
    xds = [
        nc.dram_tensor(f"x{k}d", [128, 2, n], BF16, kind="ExternalInput")
        for k, n in enumerate(XPN)
    ]
    out = nc.dram_tensor("out", [BLOC, COUT, OH, OW], BF16, kind="ExternalOutput")

    with TileContext(nc) as tc:
        with (
            tc.tile_pool(name="consts", bufs=1) as consts,
            tc.tile_pool(name="work", bufs=2) as work,
            tc.tile_pool(name="stage", bufs=2) as stage,
            tc.tile_pool(name="ps", bufs=2, space="PSUM") as pspool,
            tc.tile_pool(name="psw", bufs=1, space="PSUM") as pswpool,
            tc.tile_pool(name="psconv", bufs=2, space="PSUM") as psconv,
        ):
            # ---------- input DMAs, one ring, priority order ----------
            blob = consts.tile([128, NBLOB], F32, tag="blob")
            nc.sync.dma_start(out=blob[:], in_=blobd[:])
            cwp2 = consts.tile([128, NB, NTAP, COUT], BF16, tag="cwp2")
            for lo, hi in CWP_GROUPS:
                nc.sync.dma_start(out=cwp2[:, lo:hi], in_=cwpd[:, lo:hi])
            xsb = []
            for k, n in enumerate(XPN):
                t = consts.tile([128, 2, n], BF16, tag=f"xs{k}", name=f"xs{k}")
                nc.sync.dma_start(out=t[:], in_=xds[k][:])
                xsb.append(t)

            # views into the blob
            rvt = blob[:, BL_RVT[0] : BL_RVT[1]].bitcast(BF16).rearrange(
                "p (c b) -> p c b", b=BLOC
            )
            w1sb = blob[:, BL_W1[0] : BL_W1[1]].bitcast(BF16).rearrange(
                "p (c m) -> p c m", m=HID
            )
            w2sb = blob[:, BL_W2[0] : BL_W2[1]]
            b1sb = blob[:, BL_B1[0] : BL_B1[1]]
            b2sb = blob[0:EDIM, BL_B2[0] : BL_B2[1]]
            embnt = blob[0:EDIM, BL_EMB[0] : BL_EMB[1]]
            cb2 = blob[:, BL_CB[0] : BL_CB[1]]
            identb = blob[:, BL_ID[0] : BL_ID[1]].bitcast(BF16)

            # sel masks (in blob): j0=(b0|b2), j1=(b3|b1) for w_eff;
            # j2=(b0|b1), j3=(b2|b3) for per-bank bias/scale broadcast
            selsb = blob[0:BLOC, BL_SEL[0] : BL_SEL[1]].rearrange(
                "p (j m) -> p j m", m=128
            )
            ones64 = consts.tile([EDIM, 1], F32, tag="ones64")
            nc.vector.memset(ones64[:], 1.0)

            # ---------- routing MLP (f32 except the big W1 matmul) ----------
            h1 = pspool.tile([HID, BLOC], F32, tag="small")
            for c in range(4):
                nc.tensor.matmul(
                    h1[:], w1sb[:, c, :], rvt[:, c, :], start=(c == 0), stop=(c == 3)
                )
            h1r = work.tile([HID, BLOC], F32, tag="h1r")
            nc.scalar.activation(
                out=h1r[:], in_=h1[:], func=AF.Relu, bias=b1sb, scale=1.0
            )
            rps = pspool.tile([EDIM, BLOC], F32, tag="small")
            nc.tensor.matmul(rps[:], w2sb, h1r[:], start=True, stop=True)
            rsb = work.tile([EDIM, BLOC], F32, tag="rsb")
            nc.scalar.activation(
                out=rsb[:], in_=rps[:], func=AF.Identity, bias=b2sb, scale=1.0
            )

            # 1/||r|| (emb pre-normalized on host)
            rsq = work.tile([EDIM, BLOC], F32, tag="rsq")
            nc.vector.tensor_mul(rsq[:], rsb[:], rsb[:])
            nsq = pspool.tile([BLOC, 1], F32, tag="small")
            nc.tensor.matmul(nsq[:], rsq[:], ones64[:], start=True, stop=True)
            rln = work.tile([BLOC, 1], F32, tag="rln")
            nc.scalar.activation(out=rln[:], in_=nsq[:], func=AF.Ln)
            rinv = work.tile([BLOC, 1], F32, tag="rinv")
            nc.scalar.activation(out=rinv[:], in_=rln[:], func=AF.Exp, scale=-0.5)

            # e = exp(cos) directly: cos in [-1,1], no max-subtraction needed
            simps = pspool.tile([BLOC, NB], F32, tag="small")
            nc.tensor.matmul(simps[:], rsb[:], embnt, start=True, stop=True)
            ex = work.tile([BLOC, NB], F32, tag="ex")
            nc.scalar.activation(out=ex[:], in_=simps[:], func=AF.Exp, scale=rinv[:])

            wfbc = []
            for j in range(2):
                ps = pspool.tile([128, NB], F32, tag="small")
                nc.tensor.matmul(ps[:], selsb[:, j, :], ex[:], start=True, stop=True)
                t = work.tile([128, NB], F32, tag=f"wfbc{j}")
                nc.scalar.copy(out=t[:], in_=ps[:])
                wfbc.append(t)

            # softmax denom (does not block w_eff): sinv = 1/sum(e),
            # broadcast per output bank together with the bias weights
            s = work.tile([BLOC, 1], F32, tag="s")
            nc.vector.tensor_reduce(s[:], ex[:], axis=AX.X, op=ALU.add)
            sinv = work.tile([BLOC, 1], F32, tag="sinv")
            nc.vector.reciprocal(sinv[:], s[:])
            exs = work.tile([BLOC, NB + 1], F32, tag="exs")
            nc.vector.tensor_copy(out=exs[:, 0:NB], in_=ex[:])
            nc.vector.tensor_copy(out=exs[:, NB : NB + 1], in_=sinv[:])
            sinvbc = []
            beff = []
            for k, j in enumerate((2, 3)):
                ps = pspool.tile([128, NB + 1], F32, tag="small")
                nc.tensor.matmul(ps[:], selsb[:, j, :], exs[:], start=True, stop=True)
                t = work.tile([128, NB + 1], F32, tag=f"wfbcb{j}")
                nc.scalar.copy(out=t[:], in_=ps[:])
                sv = t[:, NB : NB + 1]
                sinvbc.append(sv)
                junk = work.tile([128, NB], F32, tag="bjunk")
                acc = work.tile([128, 1], F32, tag=f"bacc{j}")
                nc.vector.scalar_tensor_tensor(
                    out=junk[:], in0=t[:, 0:NB], scalar=1.0, in1=cb2,
                    op0=ALU.mult, op1=ALU.mult, accum_out=acc[:],
                )
                bt = work.tile([128, 1], F32, tag=f"beff{j}")
                nc.vector.tensor_mul(bt[:], acc[:], sv)
                beff.append(bt)

            # ---------- w_eff (bf16) ----------
            # weff[p, c, t, m]: c=0 -> (b0|b2), c=1 -> (b3|b1)
            weff = consts.tile([128, 2, NTAP, COUT], BF16, tag="weff")

            # c=0 on PE: accumulate diag(wfbc0[:,n]) @ cwp_n in PSUM.
            # diag matrices built on ACT from the identity in the blob.
            diags = []
            for n in range(NB):
                dg = consts.tile([128, 128], BF16, tag=f"diag{n}", name=f"diag{n}")
                nc.scalar.activation(
                    out=dg[:], in_=identb, func=AF.Copy,
                    scale=wfbc[0][:, n : n + 1],
                )
                diags.append(dg)
            psw8 = pswpool.tile([128, 512], F32, tag="psw8")
            psw1 = pswpool.tile([128, COUT], F32, tag="psw1")
            for n in range(NB):
                st_, sp = (n == 0), (n == NB - 1)
                nc.tensor.matmul(
                    psw8[:], diags[n][:], cwp2[:, n, 0:8, :], start=st_, stop=sp
                )
                nc.tensor.matmul(
                    psw1[:], diags[n][:], cwp2[:, n, 8, :], start=st_, stop=sp
                )
            nc.scalar.activation(
                out=weff[:, 0, 0:8, :],
                in_=psw8[:].rearrange("p (t m) -> p t m", m=COUT),
                func=AF.Copy,
            )
            nc.scalar.activation(out=weff[:, 0, 8, :], in_=psw1[:], func=AF.Copy)

            # c=1 on DVE: tensor_scalar products + tensor_tensor adds;
            # the last two products go to ACT (free after the diags)
            nc.vector.tensor_scalar_mul(
                out=weff[:, 1], in0=cwp2[:, 0], scalar1=wfbc[1][:, 0:1]
            )
            prods = [
                consts.tile([128, NTAP, COUT], BF16, tag=f"prod{k}", name=f"prod{k}")
                for k in range(2)
            ]
            for n in range(1, NB):
                p = prods[n % 2]
                if n >= NB - 2:
                    nc.scalar.activation(
                        out=p[:], in_=cwp2[:, n], func=AF.Copy,
                        scale=wfbc[1][:, n : n + 1],
                    )
                else:
                    nc.vector.tensor_scalar_mul(
                        out=p[:], in0=cwp2[:, n], scalar1=wfbc[1][:, n : n + 1]
                    )
                nc.vector.tensor_tensor(
                    out=weff[:, 1], in0=weff[:, 1], in1=p[:], op=ALU.add
                )

            # ---------- conv: 7 chunks x 9 taps x 4 quadrant MMs ----------
            # quadrants: b0=(0,0) psA-low, b1=(64,64) psA-high,
            #            b2=(64,0) psB-low, b3=(0,64) psB-high
            gtile = {}
            g_of_chunk = {}
            for gi, (clo, chi) in enumerate(OGROUPS):
                for ch in range(clo, chi):
                    g_of_chunk[ch] = (gi, clo, chi)
            for ch in range(NCHUNK):
                h0 = ch * CHUNK_ROWS
                xs = xsb[ch // 2]
                base = (h0 - XPIECES[ch // 2][0]) * W
                psA = psconv.tile([128, NFREE], F32, tag="A")
                psB = psconv.tile([128, NFREE], F32, tag="B")
                for t in range(NTAP):
                    off = base + TAP_OFF[t]
                    st_, sp = (t == 0), (t == NTAP - 1)
                    nc.tensor.matmul(
                        psA[0:64, :], weff[0:64, 0, t, :],
                        xs[0:64, 0, off : off + NFREE], start=st_, stop=sp,
                    )
                    nc.tensor.matmul(
                        psA[64:128, :], weff[64:128, 1, t, :],
                        xs[64:128, 0, off : off + NFREE], start=st_, stop=sp,
                    )
                    nc.tensor.matmul(
                        psB[0:64, :], weff[64:128, 0, t, :],
                        xs[64:128, 1, off : off + NFREE], start=st_, stop=sp,
                    )
                    nc.tensor.matmul(
                        psB[64:128, :], weff[0:64, 1, t, :],
                        xs[0:64, 1, off : off + NFREE], start=st_, stop=sp,
                    )
                # stage: out = psum * sinv + bias, f32->bf16, trim to 56 cols
                # bankA on ACT, bankB on DVE so neither engine paces the PE
                gi, clo, chi = g_of_chunk[ch]
                rows = (chi - clo) * CHUNK_ROWS
                r = (ch - clo) * CHUNK_ROWS
                for bi, ps in ((0, psA), (1, psB)):
                    key = (bi, gi)
                    if key not in gtile:
                        stile = stage.tile(
                            [128, rows, OW], BF16, tag=f"st{bi}_{rows}",
                            name=f"st{bi}g{gi}",
                        )
                        gtile[key] = stile
                    stile = gtile[key]
                    psv = ps[:].rearrange("p (r w) -> p r w", w=W)[:, :, 0:OW]
                    if bi == 0:
                        nc.scalar.activation(
                            out=stile[:, r : r + CHUNK_ROWS, :], in_=psv,
                            func=AF.Identity, bias=beff[bi][:], scale=sinvbc[bi],
                        )
                    else:
                        nc.vector.tensor_scalar(
                            out=stile[:, r : r + CHUNK_ROWS, :], in0=psv,
                            scalar1=sinvbc[bi], scalar2=beff[bi][:],
                            op0=ALU.mult, op1=ALU.add,
                        )
                    if ch == chi - 1:  # group complete -> DMA out (2 rings)
                        gh0 = clo * CHUNK_ROWS
                        eng = nc.gpsimd if gi % 2 == 0 else nc.sync
                        for half in range(2):
                            b = 2 * bi + half
                            eng.dma_start(
                                out=out[b, :, gh0 : gh0 + rows, :],
                                in_=stile[64 * half : 64 * half + 64],
                            )

    fix_sync_waits(nc)
    return nc


_NC = None


def _get_nc():
    global _NC
    if _NC is None:
        _NC = build()
    return _NC


def make_in_maps(inputs):
    bf16 = ml_dtypes.bfloat16

    def asf32(a):
        return np.ascontiguousarray(np.asarray(a, dtype=np.float32))

    def pack_bf16(a):
        # bf16 array -> f32-typed raw columns for the blob
        a = np.ascontiguousarray(a.astype(bf16))
        return a.reshape(a.shape[0], -1).view(np.float32)

    x = asf32(inputs["x"])
    rvec = asf32(inputs["routing_vector"])
    W1 = asf32(inputs["W1"])
    emb = asf32(inputs["emb"])
    conv_w = asf32(inputs["conv_w"])
    conv_b = asf32(inputs["conv_b"])

    embn = emb / (np.linalg.norm(emb, axis=-1, keepdims=True) + 1e-8)

    # conv_w[n, co, ci, ky, kx] -> [ci(dup 128), n, tap, co] bf16
    cwp = conv_w.transpose(2, 0, 3, 4, 1).reshape(CIN, NB, NTAP, COUT)
    cwpd = np.ascontiguousarray(np.concatenate([cwp, cwp], axis=0).astype(bf16))

    blob_common = np.zeros((128, NBLOB), np.float32)
    # W1 [512,128] -> [128, 4, 128] bf16
    w1p = W1.reshape(4, 128, HID).transpose(1, 0, 2)
    blob_common[:, BL_W1[0] : BL_W1[1]] = pack_bf16(w1p)
    blob_common[:, BL_W2[0] : BL_W2[1]] = asf32(inputs["W2"])
    blob_common[:, BL_B1[0] : BL_B1[1]] = asf32(inputs["b1"]).reshape(HID, 1)
    blob_common[0:EDIM, BL_B2[0] : BL_B2[1]] = asf32(inputs["b2"]).reshape(EDIM, 1)
    blob_common[0:EDIM, BL_EMB[0] : BL_EMB[1]] = embn.T
    blob_common[:, BL_CB[0] : BL_CB[1]] = np.tile(conv_b.T, (2, 1))
    blob_common[:, BL_ID[0] : BL_ID[1]] = pack_bf16(np.eye(128, dtype=np.float32))
    selm = np.zeros((BLOC, 4, 128), np.float32)
    for j, (blo, bhi) in enumerate(((0, 2), (3, 1), (0, 1), (2, 3))):
        selm[blo, j, 0:64] = 1.0
        selm[bhi, j, 64:128] = 1.0
    blob_common[0:BLOC, BL_SEL[0] : BL_SEL[1]] = selm.reshape(BLOC, 512)

    in_maps = []
    for c in range(NCORES):
        xs = x[BLOC * c : BLOC * (c + 1)].reshape(BLOC, CIN, HW)
        # x[p, j, i]: p<64: (j0: b0, j1: b3); p>=64: (j0: b1, j1: b2)
        xa = np.zeros((128, 2, HWP), np.float32)
        xa[0:64, 0, 0:HW] = xs[0]
        xa[64:128, 0, 0:HW] = xs[1]
        xa[64:128, 1, 0:HW] = xs[2]
        xa[0:64, 1, 0:HW] = xs[3]
        blob = blob_common.copy()
        rvs = rvec[BLOC * c : BLOC * (c + 1)]                # [4, 512]
        rvt = rvs.T.reshape(4, 128, BLOC).transpose(1, 0, 2)  # [128, 4, 4]
        blob[:, BL_RVT[0] : BL_RVT[1]] = pack_bf16(rvt)
        m = {"blobd": blob, "cwpd": cwpd}
        for k, (r0, nr) in enumerate(XPIECES):
            a = r0 * W
            m[f"x{k}d"] = np.ascontiguousarray(
                xa[:, :, a : a + XPN[k]].astype(bf16)
            )
        in_maps.append(m)
    return in_maps


def kernel(**inputs):
    from concourse.bass_utils import run_bass_kernel_spmd

    nc = _get_nc()
    in_maps = make_in_maps(inputs)
    res = run_bass_kernel_spmd(nc, in_maps, core_ids=list(range(NCORES)))
    return np.concatenate(
        [np.asarray(r["out"]).astype(np.float32) for r in res.results], axis=0
    )


# revision 28
# speedup vs baseline: 2.0354x; 1.0974x over previous
"""MoE routing layer on 8 Trainium2 NeuronCores (data-parallel over batch).

Per core (4 samples):
  routing MLP -> exp(cosine sim vs embeddings) -> unnormalized weights
  e[4,10]; w_eff[b] = sum_n e[b,n] * conv_w[n] (conv linear in weights);
  conv = 9 shifted bf16 matmuls over the flat 58-wide grid, all 4
  samples concurrent via 4-quadrant PE tiling; softmax normalization
  (1/sum e) and conv bias fold into the PSUM->SBUF staging op.

w_eff is built two ways concurrently: col-half 0 on the PE as a chain
of diagonal-stationary matmuls accumulating in PSUM (diag(wfbc_n) @
cwp_n), col-half 1 on DVE as tensor_scalar products + tensor_tensor
adds (ACT helps with late products). All heavy data is bf16; PSUM is
fp32. Host pre-packs: x in 4 row-band pieces (big DMA descriptors),
small tensors in one blob DMA, rv transposed, emb normalized,
conv weights as [ci-dup128, expert, tap, cout].
"""
import sys

sys.path.insert(0, "/opt/trn_rl_repo")

import ml_dtypes
import numpy as np

import concourse.bass as bass
import concourse.mybir as mybir
from concourse.tile import TileContext

F32 = mybir.dt.float32
BF16 = mybir.dt.bfloat16
AF = mybir.ActivationFunctionType
ALU = mybir.AluOpType
AX = mybir.AxisListType

NCORES = 8
BLOC = 4           # samples per core
CIN = 64
COUT = 64
H = W = 58
HW = H * W         # 3364
HWP = HW + 4
OH = OW = 56
NB = 10            # experts
EDIM = 64
RSIZE = 512
HID = 128
NTAP = 9
CHUNK_ROWS = 8
NCHUNK = 7
NFREE = CHUNK_ROWS * W  # 464 <= 512 (one PSUM bank)
TAP_OFF = [dy * W + dx for dy in range(3) for dx in range(3)]
# x pieces: (first input row, n rows). chunk ch (input rows 8ch..8ch+9)
# reads piece ch//2; pieces overlap by 2 rows; each padded +4 elems
XPIECES = [(0, 18), (16, 18), (32, 18), (48, 10)]
XPN = [r * W + 4 for _, r in XPIECES]
CWP_GROUPS = [(0, 5), (5, 10)]
OGROUPS = [(0, 2), (2, 4), (4, 6), (6, 7)]
# blob layout in fp32 columns: name -> (start, cols)
BL_RVT = (0, 8)        # bf16 [128, 4, 4]
BL_W1 = (8, 264)       # bf16 [128, 4, 128]
BL_W2 = (264, 296)     # bf16 [128, 64]
BL_B1 = (296, 297)     # f32 [128, 1]
BL_B2 = (297, 298)     # f32 [64, 1]
BL_EMB = (298, 308)    # f32 [64, 10]
BL_CB = (308, 318)     # f32 [128, 10]
BL_ID = (318, 382)     # bf16 [128, 128] identity
NBLOB = 382


def fix_sync_waits(nc, cap=2):
    """This walrus build allows at most `cap` sem waits per instruction.
    Splice same-engine NoOps carrying the excess waits right before any
    over-subscribed instruction (waits happen earlier => same semantics)."""
    uid = [0]
    for f in nc.m.functions:
        for blk in f.blocks:
            insts = blk.instructions  # live list
            i = 0
            while i < len(insts):
                inst = insts[i]
                si = inst.sync_info
                waits = list(si.on_wait) if si and si.on_wait else []
                icap = 1
                if len(waits) <= icap:
                    i += 1
                    continue
                keep, excess = waits[-icap:], waits[:-icap]
                for k in range(0, len(excess), icap):
                    nop = mybir.InstNoOp(
                        name=f"{inst.name}-wsplit{uid[0]}", ins=[], outs=[]
                    )
                    uid[0] += 1
                    nop.engine = inst.engine
                    nop.sync_info = mybir.SyncInfo(
                        on_wait=excess[k : k + icap], on_update=[]
                    )
                    nc.register_instruction(nop, overwrite=True)
                    insts.insert(i, nop)
                    i += 1
                inst.sync_info = mybir.SyncInfo(
                    on_wait=keep,
                    on_update=list(si.on_update) if si and si.on_update else [],
                )
                i += 1
    return nc


def build():
    nc = bass.Bass()
    # partition layout p (all [128,...] tensors): p<64 -> ci=p, samples
    # {j0: b0, j1: b3}; p>=64 -> ci=p-64, samples {j0: b1, j1: b2}
    blobd = nc.dram_tensor("blobd", [128, NBLOB], F32, kind="ExternalInput")
    seld = nc.dram_tensor("seld", [BLOC, 4, 128], BF16, kind="ExternalInput")
    cwpd = nc.dram_tensor("cwpd", [128, NB, NTAP, COUT], BF16, kind="ExternalInput")
    xds = [
        nc.dram_tensor(f"x{k}d", [128, 2, n], BF16, kind="ExternalInput")
        for k, n in enumerate(XPN)
    ]
    out = nc.dram_tensor("out", [BLOC, COUT, OH, OW], BF16, kind="ExternalOutput")

    with TileContext(nc) as tc:
        with (
            tc.tile_pool(name="consts", bufs=1) as consts,
            tc.tile_pool(name="work", bufs=2) as work,
            tc.tile_pool(name="stage", bufs=2) as stage,
            tc.tile_pool(name="ps", bufs=1, space="PSUM") as pspool,
            tc.tile_pool(name="psw", bufs=1, space="PSUM") as pswpool,
            tc.tile_pool(name="psconv", bufs=2, space="PSUM") as psconv,
        ):
            # preload the ACT function table (1.3us) before any real work
            tbl = work.tile([1, 1], F32, tag="tbl")
            nc.vector.memset(tbl[:], 1.0)
            tbl2 = work.tile([1, 1], F32, tag="tbl2")
            nc.scalar.activation(out=tbl2[:], in_=tbl[:], func=AF.Exp)

            # ---------- input DMAs, one ring, priority order ----------
            blob = consts.tile([128, NBLOB], F32, tag="blob")
            nc.sync.dma_start(out=blob[:], in_=blobd[:])
            selsb = consts.tile([BLOC, 4, 128], BF16, tag="selsb")
            nc.sync.dma_start(out=selsb[:], in_=seld[:])
            cwp2 = consts.tile([128, NB, NTAP, COUT], BF16, tag="cwp2")
            for lo, hi in CWP_GROUPS:
                nc.sync.dma_start(out=cwp2[:, lo:hi], in_=cwpd[:, lo:hi])
            xsb = []
            for k, n in enumerate(XPN):
                t = consts.tile([128, 2, n], BF16, tag=f"xs{k}", name=f"xs{k}")
                nc.sync.dma_start(out=t[:], in_=xds[k][:])
                xsb.append(t)

            # views into the blob
            rvt = blob[:, BL_RVT[0] : BL_RVT[1]].bitcast(BF16).rearrange(
                "p (c b) -> p c b", b=BLOC
            )
            w1sb = blob[:, BL_W1[0] : BL_W1[1]].bitcast(BF16).rearrange(
                "p (c m) -> p c m", m=HID
            )
            w2sb = blob[:, BL_W2[0] : BL_W2[1]].bitcast(BF16)
            b1sb = blob[:, BL_B1[0] : BL_B1[1]]
            b2sb = blob[0:EDIM, BL_B2[0] : BL_B2[1]]
            embnt = blob[0:EDIM, BL_EMB[0] : BL_EMB[1]]
            cb2 = blob[:, BL_CB[0] : BL_CB[1]]
            identb = blob[:, BL_ID[0] : BL_ID[1]].bitcast(BF16)
            ones64 = consts.tile([EDIM, 1], F32, tag="ones64")
            nc.vector.memset(ones64[:], 1.0)

            # ---------- routing MLP (f32 except the big W1 matmul) ----------
            h1 = pspool.tile([HID, BLOC], F32, tag="small")
            for c in range(4):
                nc.tensor.matmul(
                    h1[:], w1sb[:, c, :], rvt[:, c, :], start=(c == 0), stop=(c == 3)
                )
            h1r = work.tile([HID, BLOC], BF16, tag="h1r")
            nc.scalar.activation(
                out=h1r[:], in_=h1[:], func=AF.Relu, bias=b1sb, scale=1.0
            )
            rps = pspool.tile([EDIM, BLOC], F32, tag="small")
            nc.tensor.matmul(rps[:], w2sb, h1r[:], start=True, stop=True)
            rsb = work.tile([EDIM, BLOC], F32, tag="rsb")
            nc.scalar.activation(
                out=rsb[:], in_=rps[:], func=AF.Identity, bias=b2sb, scale=1.0
            )

            # 1/||r|| (emb pre-normalized on host)
            rsq = work.tile([EDIM, BLOC], F32, tag="rsq")
            nc.vector.tensor_mul(rsq[:], rsb[:], rsb[:])
            nsq = pspool.tile([BLOC, 1], F32, tag="small")
            nc.tensor.matmul(nsq[:], rsq[:], ones64[:], start=True, stop=True)
            rln = work.tile([BLOC, 1], F32, tag="rln")
            nc.scalar.activation(out=rln[:], in_=nsq[:], func=AF.Ln)
            rinv = work.tile([BLOC, 1], F32, tag="rinv")
            nc.scalar.activation(out=rinv[:], in_=rln[:], func=AF.Exp, scale=-0.5)

            # e = exp(cos) directly: cos in [-1,1], no max-subtraction needed
            simps = pspool.tile([BLOC, NB], F32, tag="small")
            nc.tensor.matmul(simps[:], rsb[:], embnt, start=True, stop=True)
            ex = work.tile([BLOC, NB], BF16, tag="ex")
            nc.scalar.activation(out=ex[:], in_=simps[:], func=AF.Exp, scale=rinv[:])

            # w_eff broadcast weights first (j0/j1 gate the diag chains)
            wfbc = []
            for j in range(2):
                ps = pspool.tile([128, NB], F32, tag="small")
                nc.tensor.matmul(ps[:], selsb[:, j, :], ex[:], start=True, stop=True)
                t = work.tile([128, NB], F32, tag=f"wfbc{j}")
                nc.scalar.copy(out=t[:], in_=ps[:])
                wfbc.append(t)

            # softmax denom: sinv = 1/sum(e); broadcast weights+sinv per
            # output bank in one bf16 matmul per selector
            s = work.tile([BLOC, 1], F32, tag="s")
            nc.vector.tensor_reduce(s[:], ex[:], axis=AX.X, op=ALU.add)
            sinv = work.tile([BLOC, 1], F32, tag="sinv")
            nc.vector.reciprocal(sinv[:], s[:])
            exs = work.tile([BLOC, NB + 1], BF16, tag="exs")
            nc.vector.tensor_copy(out=exs[:, 0:NB], in_=ex[:])
            nc.vector.tensor_copy(out=exs[:, NB : NB + 1], in_=sinv[:])
            sinvbc = []
            for j in (2, 3):
                ps = pspool.tile([128, NB + 1], F32, tag="small")
                nc.tensor.matmul(ps[:], selsb[:, j, :], exs[:], start=True, stop=True)
                t = work.tile([128, NB + 1], F32, tag=f"wfbc{j}")
                nc.scalar.copy(out=t[:], in_=ps[:])
                wfbc.append(t)
                sinvbc.append(t[:, NB : NB + 1])
            beff = []
            for k, j in enumerate((2, 3)):
                junk = work.tile([128, NB], F32, tag="bjunk")
                acc = work.tile([128, 1], F32, tag=f"bacc{j}")
                nc.vector.scalar_tensor_tensor(
                    out=junk[:], in0=wfbc[j][:, 0:NB], scalar=1.0, in1=cb2,
                    op0=ALU.mult, op1=ALU.mult, accum_out=acc[:],
                )
                bt = work.tile([128, 1], F32, tag=f"beff{j}")
                nc.vector.tensor_mul(bt[:], acc[:], sinvbc[k])
                beff.append(bt)

            # ---------- w_eff (bf16), both col-halves on the PE ----------
            # weff[p, c, t, m]: c=0 -> (b0|b2), c=1 -> (b3|b1).
            # Per expert: accumulate diag(wfbc_c[:,n]) @ cwp_n in PSUM
            # (a diagonal stationary matrix scales each partition row).
            # Diags are built on DVE; this also keeps the PE warm (HAM)
            # together with explicit warmup matmuls beforehand.
            weff = consts.tile([128, 2, NTAP, COUT], BF16, tag="weff")
            psw8a = pswpool.tile([128, 512], F32, tag="psw8a")
            psw8b = pswpool.tile([128, 512], F32, tag="psw8b")
            psw1ab = pswpool.tile([128, 2 * COUT], F32, tag="psw1ab")
            wrhs = w1sb.rearrange("p c m -> p (c m)")
            for _ in range(8):
                nc.tensor.matmul(psw8a[:], identb, wrhs, start=True, stop=True)
            diags = {}
            for c in range(2):
                for n in range(NB):
                    dg = consts.tile(
                        [128, 128], BF16, tag=f"diag{c}{n}", name=f"diag{c}{n}"
                    )
                    nc.vector.tensor_scalar_mul(
                        out=dg[:], in0=identb, scalar1=wfbc[c][:, n : n + 1]
                    )
                    diags[(c, n)] = dg
            for c, psw8 in ((0, psw8a), (1, psw8b)):
                for n in range(NB):
                    st_, sp = (n == 0), (n == NB - 1)
                    nc.tensor.matmul(
                        psw8[:], diags[(c, n)][:], cwp2[:, n, 0:8, :],
                        start=st_, stop=sp,
                    )
                    nc.tensor.matmul(
                        psw1ab[:, 64 * c : 64 * c + 64], diags[(c, n)][:],
                        cwp2[:, n, 8, :], start=st_, stop=sp,
                    )
                if c == 0:  # copies: c0 on ACT, c1 on DVE
                    nc.scalar.activation(
                        out=weff[:, 0, 0:8, :],
                        in_=psw8[:].rearrange("p (t m) -> p t m", m=COUT),
                        func=AF.Copy,
                    )
                    nc.scalar.activation(
                        out=weff[:, 0, 8, :], in_=psw1ab[:, 0:64], func=AF.Copy
                    )
                else:
                    nc.vector.tensor_copy(
                        out=weff[:, 1, 0:8, :],
                        in_=psw8[:].rearrange("p (t m) -> p t m", m=COUT),
                    )
                    nc.vector.tensor_copy(
                        out=weff[:, 1, 8, :], in_=psw1ab[:, 64:128]
                    )

            # ---------- conv: 7 chunks x 9 taps x 4 quadrant MMs ----------
            # quadrants: b0=(0,0) psA-low, b1=(64,64) psA-high,
            #            b2=(64,0) psB-low, b3=(0,64) psB-high
            gtile = {}
            g_of_chunk = {}
            for gi, (clo, chi) in enumerate(OGROUPS):
                for ch in range(clo, chi):
                    g_of_chunk[ch] = (gi, clo, chi)
            for ch in range(NCHUNK):
                h0 = ch * CHUNK_ROWS
                xs = xsb[ch // 2]
                base = (h0 - XPIECES[ch // 2][0]) * W
                psA = psconv.tile([128, NFREE], F32, tag="A")
                psB = psconv.tile([128, NFREE], F32, tag="B")
                for t in range(NTAP):
                    off = base + TAP_OFF[t]
                    st_, sp = (t == 0), (t == NTAP - 1)
                    nc.tensor.matmul(
                        psA[0:64, :], weff[0:64, 0, t, :],
                        xs[0:64, 0, off : off + NFREE], start=st_, stop=sp,
                    )
                    nc.tensor.matmul(
                        psA[64:128, :], weff[64:128, 1, t, :],
                        xs[64:128, 0, off : off + NFREE], start=st_, stop=sp,
                    )
                    nc.tensor.matmul(
                        psB[0:64, :], weff[64:128, 0, t, :],
                        xs[64:128, 1, off : off + NFREE], start=st_, stop=sp,
                    )
                    nc.tensor.matmul(
                        psB[64:128, :], weff[0:64, 1, t, :],
                        xs[0:64, 1, off : off + NFREE], start=st_, stop=sp,
                    )
                # stage: out = psum * sinv + bias, f32->bf16, trim to 56 cols
                # bankA on ACT, bankB on DVE so neither engine paces the PE
                gi, clo, chi = g_of_chunk[ch]
                rows = (chi - clo) * CHUNK_ROWS
                r = (ch - clo) * CHUNK_ROWS
                for bi, ps in ((0, psA), (1, psB)):
                    key = (bi, gi)
                    if key not in gtile:
                        stile = stage.tile(
                            [128, rows, OW], BF16, tag=f"st{bi}_{rows}",
                            name=f"st{bi}g{gi}",
                        )
                        gtile[key] = stile
                    stile = gtile[key]
                    psv = ps[:].rearrange("p (r w) -> p r w", w=W)[:, :, 0:OW]
                    if bi == 0:
                        nc.scalar.activation(
                            out=stile[:, r : r + CHUNK_ROWS, :], in_=psv,
                            func=AF.Identity, bias=beff[bi][:], scale=sinvbc[bi],
                        )
                    else:
                        nc.vector.tensor_scalar(
                            out=stile[:, r : r + CHUNK_ROWS, :], in0=psv,
                            scalar1=sinvbc[bi], scalar2=beff[bi][:],
                            op0=ALU.mult, op1=ALU.add,
                        )
                    if ch == chi - 1:  # group complete -> DMA out (2 rings)
                        gh0 = clo * CHUNK_ROWS
                        for half in range(2):
                            b = 2 * bi + half
                            eng = nc.gpsimd if (bi + half) % 2 == 0 else nc.sync
                            eng.dma_start(
                                out=out[b, :, gh0 : gh0 + rows, :],
                                in_=stile[64 * half : 64 * half + 64],
                            )

    fix_sync_waits(nc)
    return nc


_NC = None


def _get_nc():
    global _NC
    if _NC is None:
        _NC = build()
    return _NC


def make_in_maps(inputs):
    bf16 = ml_dtypes.bfloat16

    def asf32(a):
        return np.ascontiguousarray(np.asarray(a, dtype=np.float32))

    def pack_bf16(a):
        # bf16 array -> f32-typed raw columns for the blob
        a = np.ascontiguousarray(a.astype(bf16))
        return a.reshape(a.shape[0], -1).view(np.float32)

    x = asf32(inputs["x"])
    rvec = asf32(inputs["routing_vector"])
    W1 = asf32(inputs["W1"])
    emb = asf32(inputs["emb"])
    conv_w = asf32(inputs["conv_w"])
    conv_b = asf32(inputs["conv_b"])

    embn = emb / (np.linalg.norm(emb, axis=-1, keepdims=True) + 1e-8)

    # conv_w[n, co, ci, ky, kx] -> [ci(dup 128), n, tap, co] bf16
    cwp = conv_w.transpose(2, 0, 3, 4, 1).reshape(CIN, NB, NTAP, COUT)
    cwpd = np.ascontiguousarray(np.concatenate([cwp, cwp], axis=0).astype(bf16))

    blob_common = np.zeros((128, NBLOB), np.float32)
    # W1 [512,128] -> [128, 4, 128] bf16
    w1p = W1.reshape(4, 128, HID).transpose(1, 0, 2)
    blob_common[:, BL_W1[0] : BL_W1[1]] = pack_bf16(w1p)
    blob_common[:, BL_W2[0] : BL_W2[1]] = pack_bf16(asf32(inputs["W2"]))
    blob_common[:, BL_B1[0] : BL_B1[1]] = asf32(inputs["b1"]).reshape(HID, 1)
    blob_common[0:EDIM, BL_B2[0] : BL_B2[1]] = asf32(inputs["b2"]).reshape(EDIM, 1)
    blob_common[0:EDIM, BL_EMB[0] : BL_EMB[1]] = embn.T
    blob_common[:, BL_CB[0] : BL_CB[1]] = np.tile(conv_b.T, (2, 1))
    blob_common[:, BL_ID[0] : BL_ID[1]] = pack_bf16(np.eye(128, dtype=np.float32))
    selm = np.zeros((BLOC, 4, 128), np.float32)
    for j, (blo, bhi) in enumerate(((0, 2), (3, 1), (0, 1), (2, 3))):
        selm[blo, j, 0:64] = 1.0
        selm[bhi, j, 64:128] = 1.0
    selm = np.ascontiguousarray(selm.astype(bf16))

    in_maps = []
    for c in range(NCORES):
        xs = x[BLOC * c : BLOC * (c + 1)].reshape(BLOC, CIN, HW)
        # x[p, j, i]: p<64: (j0: b0, j1: b3); p>=64: (j0: b1, j1: b2)
        xa = np.zeros((128, 2, HWP), np.float32)
        xa[0:64, 0, 0:HW] = xs[0]
        xa[64:128, 0, 0:HW] = xs[1]
        xa[64:128, 1, 0:HW] = xs[2]
        xa[0:64, 1, 0:HW] = xs[3]
        blob = blob_common.copy()
        rvs = rvec[BLOC * c : BLOC * (c + 1)]                # [4, 512]
        rvt = rvs.T.reshape(4, 128, BLOC).transpose(1, 0, 2)  # [128, 4, 4]
        blob[:, BL_RVT[0] : BL_RVT[1]] = pack_bf16(rvt)
        m = {"blobd": blob, "seld": selm, "cwpd": cwpd}
        for k, (r0, nr) in enumerate(XPIECES):
            a = r0 * W
            m[f"x{k}d"] = np.ascontiguousarray(
                xa[:, :, a : a + XPN[k]].astype(bf16)
            )
        in_maps.append(m)
    return in_maps


def kernel(**inputs):
    from concourse.bass_utils import run_bass_kernel_spmd

    nc = _get_nc()
    in_maps = make_in_maps(inputs)
    res = run_bass_kernel_spmd(nc, in_maps, core_ids=list(range(NCORES)))
    return np.concatenate(
        [np.asarray(r["out"]).astype(np.float32) for r in res.results], axis=0
    )


# revision 31
# speedup vs baseline: 2.1098x; 1.0366x over previous
"""MoE routing layer on 8 Trainium2 NeuronCores (data-parallel over batch).

Per core (4 samples):
  routing MLP -> exp(cosine sim vs embeddings) -> unnormalized weights
  e[4,10]; w_eff[b] = sum_n e[b,n] * conv_w[n] (conv linear in weights);
  conv = 9 shifted bf16 matmuls over the flat 58-wide grid, all 4
  samples concurrent via 4-quadrant PE tiling; softmax normalization
  (1/sum e) and conv bias fold into the PSUM->SBUF staging op.

w_eff is built two ways concurrently: col-half 0 on the PE as a chain
of diagonal-stationary matmuls accumulating in PSUM (diag(wfbc_n) @
cwp_n), col-half 1 on DVE as tensor_scalar products + tensor_tensor
adds (ACT helps with late products). All heavy data is bf16; PSUM is
fp32. Host pre-packs: x in 4 row-band pieces (big DMA descriptors),
small tensors in one blob DMA, rv transposed, emb normalized,
conv weights as [ci-dup128, expert, tap, cout].
"""
import sys

sys.path.insert(0, "/opt/trn_rl_repo")

import ml_dtypes
import numpy as np

import concourse.bass as bass
import concourse.mybir as mybir
from concourse.tile import TileContext

F32 = mybir.dt.float32
BF16 = mybir.dt.bfloat16
AF = mybir.ActivationFunctionType
ALU = mybir.AluOpType
AX = mybir.AxisListType

NCORES = 8
BLOC = 4           # samples per core
CIN = 64
COUT = 64
H = W = 58
HW = H * W         # 3364
HWP = HW + 4
OH = OW = 56
NB = 10            # experts
EDIM = 64
RSIZE = 512
HID = 128
NTAP = 9
CHUNK_ROWS = 8
NCHUNK = 7
NFREE = CHUNK_ROWS * W  # 464 <= 512 (one PSUM bank)
TAP_OFF = [dy * W + dx for dy in range(3) for dx in range(3)]
# x pieces: (first input row, n rows). chunk ch (input rows 8ch..8ch+9)
# reads piece ch//2; pieces overlap by 2 rows; each padded +4 elems
XPIECES = [(0, 18), (16, 18), (32, 18), (48, 10)]
XPN = [r * W + 4 for _, r in XPIECES]
CWP_GROUPS = [(0, 5), (5, 10)]
OGROUPS = [(0, 1), (1, 2), (2, 4), (4, 6), (6, 7)]
# blob layout in fp32 columns: name -> (start, cols)
BL_RVT = (0, 8)        # bf16 [128, 4, 4]
BL_W1 = (8, 264)       # bf16 [128, 4, 128]
BL_W2 = (264, 296)     # bf16 [128, 64]
BL_B1 = (296, 297)     # f32 [128, 1]
BL_B2 = (297, 298)     # f32 [64, 1]
BL_EMB = (298, 308)    # f32 [64, 10]
BL_CB = (308, 318)     # f32 [128, 10]
BL_ID = (318, 382)     # bf16 [128, 128] identity
NBLOB = 382


def fix_sync_waits(nc, cap=2):
    """This walrus build allows at most `cap` sem waits per instruction.
    Splice same-engine NoOps carrying the excess waits right before any
    over-subscribed instruction (waits happen earlier => same semantics)."""
    uid = [0]
    for f in nc.m.functions:
        for blk in f.blocks:
            insts = blk.instructions  # live list
            i = 0
            while i < len(insts):
                inst = insts[i]
                si = inst.sync_info
                waits = list(si.on_wait) if si and si.on_wait else []
                icap = 1
                if len(waits) <= icap:
                    i += 1
                    continue
                keep, excess = waits[-icap:], waits[:-icap]
                for k in range(0, len(excess), icap):
                    nop = mybir.InstNoOp(
                        name=f"{inst.name}-wsplit{uid[0]}", ins=[], outs=[]
                    )
                    uid[0] += 1
                    nop.engine = inst.engine
                    nop.sync_info = mybir.SyncInfo(
                        on_wait=excess[k : k + icap], on_update=[]
                    )
                    nc.register_instruction(nop, overwrite=True)
                    insts.insert(i, nop)
                    i += 1
                inst.sync_info = mybir.SyncInfo(
                    on_wait=keep,
                    on_update=list(si.on_update) if si and si.on_update else [],
                )
                i += 1
    return nc


def build():
    nc = bass.Bass()
    # partition layout p (all [128,...] tensors): p<64 -> ci=p, samples
    # {j0: b0, j1: b3}; p>=64 -> ci=p-64, samples {j0: b1, j1: b2}
    blobd = nc.dram_tensor("blobd", [128, NBLOB], F32, kind="ExternalInput")
    seld = nc.dram_tensor("seld", [BLOC, 4, 128], BF16, kind="ExternalInput")
    cwpd = nc.dram_tensor("cwpd", [128, NB, NTAP, COUT], BF16, kind="ExternalInput")
    xds = [
        nc.dram_tensor(f"x{k}d", [128, 2, n], BF16, kind="ExternalInput")
        for k, n in enumerate(XPN)
    ]
    out = nc.dram_tensor("out", [BLOC, COUT, OH, OW], BF16, kind="ExternalOutput")

    with TileContext(nc) as tc:
        with (
            tc.tile_pool(name="consts", bufs=1) as consts,
            tc.tile_pool(name="work", bufs=2) as work,
            tc.tile_pool(name="stage", bufs=2) as stage,
            tc.tile_pool(name="ps", bufs=2, space="PSUM") as pspool,
            tc.tile_pool(name="psw", bufs=1, space="PSUM") as pswpool,
            tc.tile_pool(name="psconv", bufs=2, space="PSUM") as psconv,
        ):
            # preload the ACT function table (1.3us) before any real work
            tbl = work.tile([1, 1], F32, tag="tbl")
            nc.vector.memset(tbl[:], 1.0)
            tbl2 = work.tile([1, 1], F32, tag="tbl2")
            nc.scalar.activation(out=tbl2[:], in_=tbl[:], func=AF.Exp)

            # ---------- input DMAs, one ring, priority order ----------
            blob = consts.tile([128, NBLOB], F32, tag="blob")
            nc.sync.dma_start(out=blob[:], in_=blobd[:])
            selsb = consts.tile([BLOC, 4, 128], BF16, tag="selsb")
            nc.sync.dma_start(out=selsb[:], in_=seld[:])
            cwp2 = consts.tile([128, NB, NTAP, COUT], BF16, tag="cwp2")
            for lo, hi in CWP_GROUPS:
                nc.sync.dma_start(out=cwp2[:, lo:hi], in_=cwpd[:, lo:hi])
            xsb = []
            for k, n in enumerate(XPN):
                t = consts.tile([128, 2, n], BF16, tag=f"xs{k}", name=f"xs{k}")
                nc.sync.dma_start(out=t[:], in_=xds[k][:])
                xsb.append(t)

            # views into the blob
            rvt = blob[:, BL_RVT[0] : BL_RVT[1]].bitcast(BF16).rearrange(
                "p (c b) -> p c b", b=BLOC
            )
            w1sb = blob[:, BL_W1[0] : BL_W1[1]].bitcast(BF16).rearrange(
                "p (c m) -> p c m", m=HID
            )
            w2sb = blob[:, BL_W2[0] : BL_W2[1]].bitcast(BF16)
            b1sb = blob[:, BL_B1[0] : BL_B1[1]]
            b2sb = blob[0:EDIM, BL_B2[0] : BL_B2[1]]
            embnt = blob[0:EDIM, BL_EMB[0] : BL_EMB[1]]
            cb2 = blob[:, BL_CB[0] : BL_CB[1]]
            identb = blob[:, BL_ID[0] : BL_ID[1]].bitcast(BF16)
            ones64 = consts.tile([EDIM, 1], F32, tag="ones64")
            nc.vector.memset(ones64[:], 1.0)

            # ---------- routing MLP (f32 except the big W1 matmul) ----------
            h1 = pspool.tile([HID, BLOC], F32, tag="small")
            for c in range(4):
                nc.tensor.matmul(
                    h1[:], w1sb[:, c, :], rvt[:, c, :], start=(c == 0), stop=(c == 3)
                )
            h1r = work.tile([HID, BLOC], BF16, tag="h1r")
            nc.scalar.activation(
                out=h1r[:], in_=h1[:], func=AF.Relu, bias=b1sb, scale=1.0
            )
            rps = pspool.tile([EDIM, BLOC], F32, tag="small")
            nc.tensor.matmul(rps[:], w2sb, h1r[:], start=True, stop=True)
            rsb = work.tile([EDIM, BLOC], F32, tag="rsb")
            nc.scalar.activation(
                out=rsb[:], in_=rps[:], func=AF.Identity, bias=b2sb, scale=1.0
            )

            # 1/||r|| (emb pre-normalized on host)
            rsq = work.tile([EDIM, BLOC], F32, tag="rsq")
            nc.vector.tensor_mul(rsq[:], rsb[:], rsb[:])
            nsq = pspool.tile([BLOC, 1], F32, tag="small")
            nc.tensor.matmul(nsq[:], rsq[:], ones64[:], start=True, stop=True)
            rln = work.tile([BLOC, 1], F32, tag="rln")
            nc.scalar.activation(out=rln[:], in_=nsq[:], func=AF.Ln)
            rinv = work.tile([BLOC, 1], F32, tag="rinv")
            nc.scalar.activation(out=rinv[:], in_=rln[:], func=AF.Exp, scale=-0.5)

            # e = exp(cos) directly: cos in [-1,1], no max-subtraction needed
            simps = pspool.tile([BLOC, NB], F32, tag="small")
            nc.tensor.matmul(simps[:], rsb[:], embnt, start=True, stop=True)
            ex = work.tile([BLOC, NB], BF16, tag="ex")
            nc.scalar.activation(out=ex[:], in_=simps[:], func=AF.Exp, scale=rinv[:])

            # w_eff broadcast weights first (j0/j1 gate the diag chains)
            wfbc = []
            for j in range(2):
                ps = pspool.tile([128, NB], F32, tag="small")
                nc.tensor.matmul(ps[:], selsb[:, j, :], ex[:], start=True, stop=True)
                t = work.tile([128, NB], F32, tag=f"wfbc{j}")
                nc.scalar.copy(out=t[:], in_=ps[:])
                wfbc.append(t)

            # ---------- w_eff (bf16) ----------
            # weff[p, c, t, m]: c=0 -> (b0|b2), c=1 -> (b3|b1).
            # Taps 0-7 per expert on the PE: accumulate diag(wfbc_c[:,n])
            # @ cwp_n[taps 0-7] in PSUM (a diagonal stationary matrix
            # scales each partition row; one N=512 matmul per expert).
            # Tap 8 via fused scalar_tensor_tensor chains on DVE. Diags
            # built on DVE (c0) and ACT (c1). Warmup matmuls un-throttle
            # the PE clock (HAM) during this phase.
            weff = consts.tile([128, 2, NTAP, COUT], BF16, tag="weff")
            psw8a = pswpool.tile([128, 512], F32, tag="psw8a")
            psw8b = pswpool.tile([128, 512], F32, tag="psw8b")
            wrhs = w1sb.rearrange("p c m -> p (c m)")
            for _ in range(8):
                nc.tensor.matmul(psw8a[:], identb, wrhs, start=True, stop=True)
            diags = {}
            for c in range(2):
                for n in range(NB):
                    dg = consts.tile(
                        [128, 128], BF16, tag=f"diag{c}{n}", name=f"diag{c}{n}"
                    )
                    if c == 0:
                        nc.vector.tensor_scalar_mul(
                            out=dg[:], in0=identb, scalar1=wfbc[c][:, n : n + 1]
                        )
                    else:
                        nc.scalar.activation(
                            out=dg[:], in_=identb, func=AF.Copy,
                            scale=wfbc[c][:, n : n + 1],
                        )
                    diags[(c, n)] = dg
            # tap-8 chains on DVE
            for c in range(2):
                nc.vector.tensor_scalar_mul(
                    out=weff[:, c, 8, :], in0=cwp2[:, 0, 8, :],
                    scalar1=wfbc[c][:, 0:1],
                )
                for n in range(1, NB):
                    nc.vector.scalar_tensor_tensor(
                        out=weff[:, c, 8, :], in0=cwp2[:, n, 8, :],
                        scalar=wfbc[c][:, n : n + 1], in1=weff[:, c, 8, :],
                        op0=ALU.mult, op1=ALU.add,
                    )
            # taps 0-7 on PE + staging copies on ACT
            for c, psw8 in ((0, psw8a), (1, psw8b)):
                for n in range(NB):
                    nc.tensor.matmul(
                        psw8[:], diags[(c, n)][:], cwp2[:, n, 0:8, :],
                        start=(n == 0), stop=(n == NB - 1),
                    )
                nc.scalar.activation(
                    out=weff[:, c, 0:8, :],
                    in_=psw8[:].rearrange("p (t m) -> p t m", m=COUT),
                    func=AF.Copy,
                )

            # per-bank bias/scale broadcast (needed only at stage time)
            s = work.tile([BLOC, 1], F32, tag="s")
            nc.vector.tensor_reduce(s[:], ex[:], axis=AX.X, op=ALU.add)
            sinv = work.tile([BLOC, 1], F32, tag="sinv")
            nc.vector.reciprocal(sinv[:], s[:])
            exs = work.tile([BLOC, NB + 1], BF16, tag="exs")
            nc.vector.tensor_copy(out=exs[:, 0:NB], in_=ex[:])
            nc.vector.tensor_copy(out=exs[:, NB : NB + 1], in_=sinv[:])
            sinvbc = []
            for j in (2, 3):
                ps = pspool.tile([128, NB + 1], F32, tag="small")
                nc.tensor.matmul(ps[:], selsb[:, j, :], exs[:], start=True, stop=True)
                t = work.tile([128, NB + 1], F32, tag=f"wfbc{j}")
                nc.scalar.copy(out=t[:], in_=ps[:])
                wfbc.append(t)
                sinvbc.append(t[:, NB : NB + 1])
            beff = []
            for k, j in enumerate((2, 3)):
                junk = work.tile([128, NB], F32, tag="bjunk")
                acc = work.tile([128, 1], F32, tag=f"bacc{j}")
                nc.vector.scalar_tensor_tensor(
                    out=junk[:], in0=wfbc[j][:, 0:NB], scalar=1.0, in1=cb2,
                    op0=ALU.mult, op1=ALU.mult, accum_out=acc[:],
                )
                bt = work.tile([128, 1], F32, tag=f"beff{j}")
                nc.vector.tensor_mul(bt[:], acc[:], sinvbc[k])
                beff.append(bt)

            # ---------- conv: 7 chunks x 9 taps x 4 quadrant MMs ----------
            # quadrants: b0=(0,0) psA-low, b1=(64,64) psA-high,
            #            b2=(64,0) psB-low, b3=(0,64) psB-high
            gtile = {}
            g_of_chunk = {}
            for gi, (clo, chi) in enumerate(OGROUPS):
                for ch in range(clo, chi):
                    g_of_chunk[ch] = (gi, clo, chi)
            for ch in range(NCHUNK):
                h0 = ch * CHUNK_ROWS
                xs = xsb[ch // 2]
                base = (h0 - XPIECES[ch // 2][0]) * W
                psA = psconv.tile([128, NFREE], F32, tag="A")
                psB = psconv.tile([128, NFREE], F32, tag="B")
                for t in range(NTAP):
                    off = base + TAP_OFF[t]
                    st_, sp = (t == 0), (t == NTAP - 1)
                    nc.tensor.matmul(
                        psA[0:64, :], weff[0:64, 0, t, :],
                        xs[0:64, 0, off : off + NFREE], start=st_, stop=sp,
                    )
                    nc.tensor.matmul(
                        psA[64:128, :], weff[64:128, 1, t, :],
                        xs[64:128, 0, off : off + NFREE], start=st_, stop=sp,
                    )
                    nc.tensor.matmul(
                        psB[0:64, :], weff[64:128, 0, t, :],
                        xs[64:128, 1, off : off + NFREE], start=st_, stop=sp,
                    )
                    nc.tensor.matmul(
                        psB[64:128, :], weff[0:64, 1, t, :],
                        xs[0:64, 1, off : off + NFREE], start=st_, stop=sp,
                    )
                # stage: out = psum * sinv + bias, f32->bf16, trim to 56 cols
                # bankA on ACT, bankB on DVE so neither engine paces the PE
                gi, clo, chi = g_of_chunk[ch]
                rows = (chi - clo) * CHUNK_ROWS
                r = (ch - clo) * CHUNK_ROWS
                for bi, ps in ((0, psA), (1, psB)):
                    key = (bi, gi)
                    if key not in gtile:
                        stile = stage.tile(
                            [128, rows, OW], BF16, tag=f"st{bi}_{rows}",
                            name=f"st{bi}g{gi}",
                        )
                        gtile[key] = stile
                    stile = gtile[key]
                    psv = ps[:].rearrange("p (r w) -> p r w", w=W)[:, :, 0:OW]
                    if bi == 0:
                        nc.scalar.activation(
                            out=stile[:, r : r + CHUNK_ROWS, :], in_=psv,
                            func=AF.Identity, bias=beff[bi][:], scale=sinvbc[bi],
                        )
                    else:
                        nc.vector.tensor_scalar(
                            out=stile[:, r : r + CHUNK_ROWS, :], in0=psv,
                            scalar1=sinvbc[bi], scalar2=beff[bi][:],
                            op0=ALU.mult, op1=ALU.add,
                        )
                    if ch == chi - 1:  # group complete -> DMA out (2 rings)
                        gh0 = clo * CHUNK_ROWS
                        for half in range(2):
                            b = 2 * bi + half
                            eng = nc.gpsimd if (bi + half) % 2 == 0 else nc.sync
                            eng.dma_start(
                                out=out[b, :, gh0 : gh0 + rows, :],
                                in_=stile[64 * half : 64 * half + 64],
                            )

    fix_sync_waits(nc)
    return nc


_NC = None


def _get_nc():
    global _NC
    if _NC is None:
        _NC = build()
    return _NC


def make_in_maps(inputs):
    bf16 = ml_dtypes.bfloat16

    def asf32(a):
        return np.ascontiguousarray(np.asarray(a, dtype=np.float32))

    def pack_bf16(a):
        # bf16 array -> f32-typed raw columns for the blob
        a = np.ascontiguousarray(a.astype(bf16))
        return a.reshape(a.shape[0], -1).view(np.float32)

    x = asf32(inputs["x"])
    rvec = asf32(inputs["routing_vector"])
    W1 = asf32(inputs["W1"])
    emb = asf32(inputs["emb"])
    conv_w = asf32(inputs["conv_w"])
    conv_b = asf32(inputs["conv_b"])

    embn = emb / (np.linalg.norm(emb, axis=-1, keepdims=True) + 1e-8)

    # conv_w[n, co, ci, ky, kx] -> [ci(dup 128), n, tap, co] bf16
    cwp = conv_w.transpose(2, 0, 3, 4, 1).reshape(CIN, NB, NTAP, COUT)
    cwpd = np.ascontiguousarray(np.concatenate([cwp, cwp], axis=0).astype(bf16))

    blob_common = np.zeros((128, NBLOB), np.float32)
    # W1 [512,128] -> [128, 4, 128] bf16
    w1p = W1.reshape(4, 128, HID).transpose(1, 0, 2)
    blob_common[:, BL_W1[0] : BL_W1[1]] = pack_bf16(w1p)
    blob_common[:, BL_W2[0] : BL_W2[1]] = pack_bf16(asf32(inputs["W2"]))
    blob_common[:, BL_B1[0] : BL_B1[1]] = asf32(inputs["b1"]).reshape(HID, 1)
    blob_common[0:EDIM, BL_B2[0] : BL_B2[1]] = asf32(inputs["b2"]).reshape(EDIM, 1)
    blob_common[0:EDIM, BL_EMB[0] : BL_EMB[1]] = embn.T
    blob_common[:, BL_CB[0] : BL_CB[1]] = np.tile(conv_b.T, (2, 1))
    blob_common[:, BL_ID[0] : BL_ID[1]] = pack_bf16(np.eye(128, dtype=np.float32))
    selm = np.zeros((BLOC, 4, 128), np.float32)
    for j, (blo, bhi) in enumerate(((0, 2), (3, 1), (0, 1), (2, 3))):
        selm[blo, j, 0:64] = 1.0
        selm[bhi, j, 64:128] = 1.0
    selm = np.ascontiguousarray(selm.astype(bf16))

    in_maps = []
    for c in range(NCORES):
        xs = x[BLOC * c : BLOC * (c + 1)].reshape(BLOC, CIN, HW)
        # x[p, j, i]: p<64: (j0: b0, j1: b3); p>=64: (j0: b1, j1: b2)
        xa = np.zeros((128, 2, HWP), np.float32)
        xa[0:64, 0, 0:HW] = xs[0]
        xa[64:128, 0, 0:HW] = xs[1]
        xa[64:128, 1, 0:HW] = xs[2]
        xa[0:64, 1, 0:HW] = xs[3]
        blob = blob_common.copy()
        rvs = rvec[BLOC * c : BLOC * (c + 1)]                # [4, 512]
        rvt = rvs.T.reshape(4, 128, BLOC).transpose(1, 0, 2)  # [128, 4, 4]
        blob[:, BL_RVT[0] : BL_RVT[1]] = pack_bf16(rvt)
        m = {"blobd": blob, "seld": selm, "cwpd": cwpd}
        for k, (r0, nr) in enumerate(XPIECES):
            a = r0 * W
            m[f"x{k}d"] = np.ascontiguousarray(
                xa[:, :, a : a + XPN[k]].astype(bf16)
            )
        in_maps.append(m)
    return in_maps


def kernel(**inputs):
    from concourse.bass_utils import run_bass_kernel_spmd

    nc = _get_nc()
    in_maps = make_in_maps(inputs)
    res = run_bass_kernel_spmd(nc, in_maps, core_ids=list(range(NCORES)))
    return np.concatenate(
        [np.asarray(r["out"]).astype(np.float32) for r in res.results], axis=0
    )


# revision 33
# speedup vs baseline: 2.1654x; 1.0264x over previous
"""MoE routing layer on 8 Trainium2 NeuronCores (data-parallel over batch).

Per core (4 samples):
  routing MLP -> exp(cosine sim vs embeddings) -> unnormalized weights
  e[4,10]; w_eff[b] = sum_n e[b,n] * conv_w[n] (conv linear in weights);
  conv = 9 shifted bf16 matmuls over the flat 58-wide grid, all 4
  samples concurrent via 4-quadrant PE tiling; softmax normalization
  (1/sum e) and conv bias fold into the PSUM->SBUF staging op.

w_eff is built two ways concurrently: col-half 0 on the PE as a chain
of diagonal-stationary matmuls accumulating in PSUM (diag(wfbc_n) @
cwp_n), col-half 1 on DVE as tensor_scalar products + tensor_tensor
adds (ACT helps with late products). All heavy data is bf16; PSUM is
fp32. Host pre-packs: x in 4 row-band pieces (big DMA descriptors),
small tensors in one blob DMA, rv transposed, emb normalized,
conv weights as [ci-dup128, expert, tap, cout].
"""
import sys

sys.path.insert(0, "/opt/trn_rl_repo")

import ml_dtypes
import numpy as np

import concourse.bass as bass
import concourse.mybir as mybir
from concourse.tile import TileContext

F32 = mybir.dt.float32
BF16 = mybir.dt.bfloat16
AF = mybir.ActivationFunctionType
ALU = mybir.AluOpType
AX = mybir.AxisListType

NCORES = 8
BLOC = 4           # samples per core
CIN = 64
COUT = 64
H = W = 58
HW = H * W         # 3364
HWP = HW + 4
OH = OW = 56
NB = 10            # experts
EDIM = 64
RSIZE = 512
HID = 128
NTAP = 9
CHUNK_ROWS = 8
NCHUNK = 7
NFREE = CHUNK_ROWS * W  # 464 <= 512 (one PSUM bank)
TAP_OFF = [dy * W + dx for dy in range(3) for dx in range(3)]
# x pieces: (first input row, n rows). chunk ch (input rows 8ch..8ch+9)
# reads piece ch//2; pieces overlap by 2 rows; each padded +4 elems
XPIECES = [(0, 18), (16, 18), (32, 18), (48, 10)]
XPN = [r * W + 4 for _, r in XPIECES]
CWP_GROUPS = [(0, 5), (5, 10)]
OGROUPS = [(0, 4), (4, 6), (6, 7)]
# blob layout in fp32 columns: name -> (start, cols)
BL_RVT = (0, 8)        # bf16 [128, 4, 4]
BL_W1 = (8, 264)       # bf16 [128, 4, 128]
BL_W2 = (264, 296)     # bf16 [128, 64]
BL_B1 = (296, 297)     # f32 [128, 1]
BL_B2 = (297, 298)     # f32 [64, 1]
BL_EMB = (298, 308)    # f32 [64, 10]
BL_CB = (308, 318)     # f32 [128, 10]
BL_ID = (318, 382)     # bf16 [128, 128] identity
NBLOB = 382


def fix_sync_waits(nc, cap=2):
    """This walrus build allows at most `cap` sem waits per instruction.
    Splice same-engine NoOps carrying the excess waits right before any
    over-subscribed instruction (waits happen earlier => same semantics)."""
    uid = [0]
    for f in nc.m.functions:
        for blk in f.blocks:
            insts = blk.instructions  # live list
            i = 0
            while i < len(insts):
                inst = insts[i]
                si = inst.sync_info
                waits = list(si.on_wait) if si and si.on_wait else []
                icap = 1
                if len(waits) <= icap:
                    i += 1
                    continue
                keep, excess = waits[-icap:], waits[:-icap]
                for k in range(0, len(excess), icap):
                    nop = mybir.InstNoOp(
                        name=f"{inst.name}-wsplit{uid[0]}", ins=[], outs=[]
                    )
                    uid[0] += 1
                    nop.engine = inst.engine
                    nop.sync_info = mybir.SyncInfo(
                        on_wait=excess[k : k + icap], on_update=[]
                    )
                    nc.register_instruction(nop, overwrite=True)
                    insts.insert(i, nop)
                    i += 1
                inst.sync_info = mybir.SyncInfo(
                    on_wait=keep,
                    on_update=list(si.on_update) if si and si.on_update else [],
                )
                i += 1
    return nc


def build():
    nc = bass.Bass()
    # partition layout p (all [128,...] tensors): p<64 -> ci=p, samples
    # {j0: b0, j1: b3}; p>=64 -> ci=p-64, samples {j0: b1, j1: b2}
    blobd = nc.dram_tensor("blobd", [128, NBLOB], F32, kind="ExternalInput")
    seld = nc.dram_tensor("seld", [BLOC, 4, 128], BF16, kind="ExternalInput")
    cwpd = nc.dram_tensor("cwpd", [128, NB, NTAP, COUT], BF16, kind="ExternalInput")
    xds = [
        nc.dram_tensor(f"x{k}d", [128, 2, n], BF16, kind="ExternalInput")
        for k, n in enumerate(XPN)
    ]
    out = nc.dram_tensor("out", [BLOC, COUT, OH, OW], BF16, kind="ExternalOutput")

    with TileContext(nc) as tc:
        with (
            tc.tile_pool(name="consts", bufs=1) as consts,
            tc.tile_pool(name="work", bufs=2) as work,
            tc.tile_pool(name="stage", bufs=2) as stage,
            tc.tile_pool(name="ps", bufs=2, space="PSUM") as pspool,
            tc.tile_pool(name="psw", bufs=1, space="PSUM") as pswpool,
            tc.tile_pool(name="psconv", bufs=2, space="PSUM") as psconv,
        ):
            # preload the ACT function table (1.3us) before any real work
            tbl = work.tile([1, 1], F32, tag="tbl")
            nc.vector.memset(tbl[:], 1.0)
            tbl2 = work.tile([1, 1], F32, tag="tbl2")
            nc.scalar.activation(out=tbl2[:], in_=tbl[:], func=AF.Exp)

            # ---------- input DMAs, one ring, priority order ----------
            blob = consts.tile([128, NBLOB], F32, tag="blob")
            nc.sync.dma_start(out=blob[:], in_=blobd[:])
            selsb = consts.tile([BLOC, 4, 128], BF16, tag="selsb")
            nc.sync.dma_start(out=selsb[:], in_=seld[:])
            cwp2 = consts.tile([128, NB, NTAP, COUT], BF16, tag="cwp2")
            for lo, hi in CWP_GROUPS:
                nc.sync.dma_start(out=cwp2[:, lo:hi], in_=cwpd[:, lo:hi])
            xsb = []
            for k, n in enumerate(XPN):
                t = consts.tile([128, 2, n], BF16, tag=f"xs{k}", name=f"xs{k}")
                nc.sync.dma_start(out=t[:], in_=xds[k][:])
                xsb.append(t)

            # views into the blob
            rvt = blob[:, BL_RVT[0] : BL_RVT[1]].bitcast(BF16).rearrange(
                "p (c b) -> p c b", b=BLOC
            )
            w1sb = blob[:, BL_W1[0] : BL_W1[1]].bitcast(BF16).rearrange(
                "p (c m) -> p c m", m=HID
            )
            w2sb = blob[:, BL_W2[0] : BL_W2[1]].bitcast(BF16)
            b1sb = blob[:, BL_B1[0] : BL_B1[1]]
            b2sb = blob[0:EDIM, BL_B2[0] : BL_B2[1]]
            embnt = blob[0:EDIM, BL_EMB[0] : BL_EMB[1]]
            cb2 = blob[:, BL_CB[0] : BL_CB[1]]
            identb = blob[:, BL_ID[0] : BL_ID[1]].bitcast(BF16)
            ones64 = consts.tile([EDIM, 1], F32, tag="ones64")
            nc.vector.memset(ones64[:], 1.0)

            # ---------- routing MLP (f32 except the big W1 matmul) ----------
            h1 = pspool.tile([HID, BLOC], F32, tag="small")
            for c in range(4):
                nc.tensor.matmul(
                    h1[:], w1sb[:, c, :], rvt[:, c, :], start=(c == 0), stop=(c == 3)
                )
            h1r = work.tile([HID, BLOC], BF16, tag="h1r")
            nc.scalar.activation(
                out=h1r[:], in_=h1[:], func=AF.Relu, bias=b1sb, scale=1.0
            )
            rps = pspool.tile([EDIM, BLOC], F32, tag="small")
            nc.tensor.matmul(rps[:], w2sb, h1r[:], start=True, stop=True)
            rsb = work.tile([EDIM, BLOC], F32, tag="rsb")
            nc.scalar.activation(
                out=rsb[:], in_=rps[:], func=AF.Identity, bias=b2sb, scale=1.0
            )

            # 1/||r|| (emb pre-normalized on host)
            rsq = work.tile([EDIM, BLOC], F32, tag="rsq")
            nc.vector.tensor_mul(rsq[:], rsb[:], rsb[:])
            nsq = pspool.tile([BLOC, 1], F32, tag="small")
            nc.tensor.matmul(nsq[:], rsq[:], ones64[:], start=True, stop=True)
            rln = work.tile([BLOC, 1], F32, tag="rln")
            nc.scalar.activation(out=rln[:], in_=nsq[:], func=AF.Ln)
            rinv = work.tile([BLOC, 1], F32, tag="rinv")
            nc.scalar.activation(out=rinv[:], in_=rln[:], func=AF.Exp, scale=-0.5)

            # e = exp(cos) directly: cos in [-1,1], no max-subtraction needed
            simps = pspool.tile([BLOC, NB], F32, tag="small")
            nc.tensor.matmul(simps[:], rsb[:], embnt, start=True, stop=True)
            ex = work.tile([BLOC, NB], BF16, tag="ex")
            nc.scalar.activation(out=ex[:], in_=simps[:], func=AF.Exp, scale=rinv[:])

            # w_eff broadcast weights first (j0/j1 gate the diag chains)
            wfbc = []
            for j in range(2):
                ps = pspool.tile([128, NB], F32, tag="small")
                nc.tensor.matmul(ps[:], selsb[:, j, :], ex[:], start=True, stop=True)
                t = work.tile([128, NB], F32, tag=f"wfbc{j}")
                nc.scalar.copy(out=t[:], in_=ps[:])
                wfbc.append(t)

            # ---------- w_eff (bf16) ----------
            # weff[p, c, t, m]: c=0 -> (b0|b2), c=1 -> (b3|b1).
            # Taps 0-7 per expert on the PE: accumulate diag(wfbc_c[:,n])
            # @ cwp_n[taps 0-7] in PSUM (a diagonal stationary matrix
            # scales each partition row; one N=512 matmul per expert).
            # Tap 8 via fused scalar_tensor_tensor chains on DVE. Diags
            # built on DVE (c0) and ACT (c1). Warmup matmuls un-throttle
            # the PE clock (HAM) during this phase.
            weff = consts.tile([128, 2, NTAP, COUT], BF16, tag="weff")
            psw8a = pswpool.tile([128, 512], F32, tag="psw8a")
            psw8b = pswpool.tile([128, 512], F32, tag="psw8b")
            wrhs = w1sb.rearrange("p c m -> p (c m)")
            for _ in range(8):
                nc.tensor.matmul(psw8a[:], identb, wrhs, start=True, stop=True)
            diags = {}
            for c in range(2):
                for n in range(NB):
                    dg = consts.tile(
                        [128, 128], BF16, tag=f"diag{c}{n}", name=f"diag{c}{n}"
                    )
                    if c == 0:
                        nc.vector.tensor_scalar_mul(
                            out=dg[:], in0=identb, scalar1=wfbc[c][:, n : n + 1]
                        )
                    else:
                        nc.scalar.activation(
                            out=dg[:], in_=identb, func=AF.Copy,
                            scale=wfbc[c][:, n : n + 1],
                        )
                    diags[(c, n)] = dg
            # tap-8 chains on DVE
            for c in range(2):
                nc.vector.tensor_scalar_mul(
                    out=weff[:, c, 8, :], in0=cwp2[:, 0, 8, :],
                    scalar1=wfbc[c][:, 0:1],
                )
                for n in range(1, NB):
                    nc.vector.scalar_tensor_tensor(
                        out=weff[:, c, 8, :], in0=cwp2[:, n, 8, :],
                        scalar=wfbc[c][:, n : n + 1], in1=weff[:, c, 8, :],
                        op0=ALU.mult, op1=ALU.add,
                    )
            # taps 0-7 on PE + staging copies on ACT
            for c, psw8 in ((0, psw8a), (1, psw8b)):
                for n in range(NB):
                    nc.tensor.matmul(
                        psw8[:], diags[(c, n)][:], cwp2[:, n, 0:8, :],
                        start=(n == 0), stop=(n == NB - 1),
                    )
                nc.scalar.activation(
                    out=weff[:, c, 0:8, :],
                    in_=psw8[:].rearrange("p (t m) -> p t m", m=COUT),
                    func=AF.Copy,
                )

            # per-bank bias/scale broadcast (needed only at stage time)
            s = work.tile([BLOC, 1], F32, tag="s")
            nc.vector.tensor_reduce(s[:], ex[:], axis=AX.X, op=ALU.add)
            sinv = work.tile([BLOC, 1], F32, tag="sinv")
            nc.vector.reciprocal(sinv[:], s[:])
            exs = work.tile([BLOC, NB + 1], BF16, tag="exs")
            nc.vector.tensor_copy(out=exs[:, 0:NB], in_=ex[:])
            nc.vector.tensor_copy(out=exs[:, NB : NB + 1], in_=sinv[:])
            sinvbc = []
            for j in (2, 3):
                ps = pspool.tile([128, NB + 1], F32, tag="small")
                nc.tensor.matmul(ps[:], selsb[:, j, :], exs[:], start=True, stop=True)
                t = work.tile([128, NB + 1], F32, tag=f"wfbc{j}")
                nc.scalar.copy(out=t[:], in_=ps[:])
                wfbc.append(t)
                sinvbc.append(t[:, NB : NB + 1])
            beff = []
            for k, j in enumerate((2, 3)):
                junk = work.tile([128, NB], F32, tag="bjunk")
                acc = work.tile([128, 1], F32, tag=f"bacc{j}")
                nc.vector.scalar_tensor_tensor(
                    out=junk[:], in0=wfbc[j][:, 0:NB], scalar=1.0, in1=cb2,
                    op0=ALU.mult, op1=ALU.mult, accum_out=acc[:],
                )
                bt = work.tile([128, 1], F32, tag=f"beff{j}")
                nc.vector.tensor_mul(bt[:], acc[:], sinvbc[k])
                beff.append(bt)

            # ---------- conv: 7 chunks x 9 taps x 4 quadrant MMs ----------
            # quadrants: b0=(0,0) psA-low, b1=(64,64) psA-high,
            #            b2=(64,0) psB-low, b3=(0,64) psB-high
            gtile = {}
            g_of_chunk = {}
            for gi, (clo, chi) in enumerate(OGROUPS):
                for ch in range(clo, chi):
                    g_of_chunk[ch] = (gi, clo, chi)
            for ch in range(NCHUNK):
                h0 = ch * CHUNK_ROWS
                xs = xsb[ch // 2]
                base = (h0 - XPIECES[ch // 2][0]) * W
                psA = psconv.tile([128, NFREE], F32, tag="A")
                psB = psconv.tile([128, NFREE], F32, tag="B")
                for t in range(NTAP):
                    off = base + TAP_OFF[t]
                    st_, sp = (t == 0), (t == NTAP - 1)
                    nc.tensor.matmul(
                        psA[0:64, :], weff[0:64, 0, t, :],
                        xs[0:64, 0, off : off + NFREE], start=st_, stop=sp,
                    )
                    nc.tensor.matmul(
                        psA[64:128, :], weff[64:128, 1, t, :],
                        xs[64:128, 0, off : off + NFREE], start=st_, stop=sp,
                    )
                    nc.tensor.matmul(
                        psB[0:64, :], weff[64:128, 0, t, :],
                        xs[64:128, 1, off : off + NFREE], start=st_, stop=sp,
                    )
                    nc.tensor.matmul(
                        psB[64:128, :], weff[0:64, 1, t, :],
                        xs[0:64, 1, off : off + NFREE], start=st_, stop=sp,
                    )
                # stage: out = psum * sinv + bias, f32->bf16, trim to 56 cols
                # bankA on ACT, bankB on DVE so neither engine paces the PE
                gi, clo, chi = g_of_chunk[ch]
                rows = (chi - clo) * CHUNK_ROWS
                r = (ch - clo) * CHUNK_ROWS
                for bi, ps in ((0, psA), (1, psB)):
                    key = (bi, gi)
                    if key not in gtile:
                        stile = stage.tile(
                            [128, rows, OW], BF16, tag=f"st{bi}_{rows}",
                            name=f"st{bi}g{gi}",
                        )
                        gtile[key] = stile
                    stile = gtile[key]
                    psv = ps[:].rearrange("p (r w) -> p r w", w=W)[:, :, 0:OW]
                    if bi == 0:
                        nc.scalar.activation(
                            out=stile[:, r : r + CHUNK_ROWS, :], in_=psv,
                            func=AF.Identity, bias=beff[bi][:], scale=sinvbc[bi],
                        )
                    else:
                        nc.vector.tensor_scalar(
                            out=stile[:, r : r + CHUNK_ROWS, :], in0=psv,
                            scalar1=sinvbc[bi], scalar2=beff[bi][:],
                            op0=ALU.mult, op1=ALU.add,
                        )
                    if ch == chi - 1:  # group complete -> DMA out (2 rings,
                        # one DMA per bank covering both samples)
                        gh0 = clo * CHUNK_ROWS
                        eng = nc.gpsimd if bi == 0 else nc.sync
                        eng.dma_start(
                            out=out[2 * bi : 2 * bi + 2, :, gh0 : gh0 + rows, :]
                            .rearrange("b c r w -> (b c) r w"),
                            in_=stile[:],
                        )

    fix_sync_waits(nc)
    return nc


_NC = None


def _get_nc():
    global _NC
    if _NC is None:
        _NC = build()
    return _NC


def make_in_maps(inputs):
    bf16 = ml_dtypes.bfloat16

    def asf32(a):
        return np.ascontiguousarray(np.asarray(a, dtype=np.float32))

    def pack_bf16(a):
        # bf16 array -> f32-typed raw columns for the blob
        a = np.ascontiguousarray(a.astype(bf16))
        return a.reshape(a.shape[0], -1).view(np.float32)

    x = asf32(inputs["x"])
    rvec = asf32(inputs["routing_vector"])
    W1 = asf32(inputs["W1"])
    emb = asf32(inputs["emb"])
    conv_w = asf32(inputs["conv_w"])
    conv_b = asf32(inputs["conv_b"])

    embn = emb / (np.linalg.norm(emb, axis=-1, keepdims=True) + 1e-8)

    # conv_w[n, co, ci, ky, kx] -> [ci(dup 128), n, tap, co] bf16
    cwp = conv_w.transpose(2, 0, 3, 4, 1).reshape(CIN, NB, NTAP, COUT)
    cwpd = np.ascontiguousarray(np.concatenate([cwp, cwp], axis=0).astype(bf16))

    blob_common = np.zeros((128, NBLOB), np.float32)
    # W1 [512,128] -> [128, 4, 128] bf16
    w1p = W1.reshape(4, 128, HID).transpose(1, 0, 2)
    blob_common[:, BL_W1[0] : BL_W1[1]] = pack_bf16(w1p)
    blob_common[:, BL_W2[0] : BL_W2[1]] = pack_bf16(asf32(inputs["W2"]))
    blob_common[:, BL_B1[0] : BL_B1[1]] = asf32(inputs["b1"]).reshape(HID, 1)
    blob_common[0:EDIM, BL_B2[0] : BL_B2[1]] = asf32(inputs["b2"]).reshape(EDIM, 1)
    blob_common[0:EDIM, BL_EMB[0] : BL_EMB[1]] = embn.T
    blob_common[:, BL_CB[0] : BL_CB[1]] = np.tile(conv_b.T, (2, 1))
    blob_common[:, BL_ID[0] : BL_ID[1]] = pack_bf16(np.eye(128, dtype=np.float32))
    selm = np.zeros((BLOC, 4, 128), np.float32)
    for j, (blo, bhi) in enumerate(((0, 2), (3, 1), (0, 1), (2, 3))):
        selm[blo, j, 0:64] = 1.0
        selm[bhi, j, 64:128] = 1.0
    selm = np.ascontiguousarray(selm.astype(bf16))

    in_maps = []
    for c in range(NCORES):
        xs = x[BLOC * c : BLOC * (c + 1)].reshape(BLOC, CIN, HW)
        # x[p, j, i]: p<64: (j0: b0, j1: b3); p>=64: (j0: b1, j1: b2)
        xa = np.zeros((128, 2, HWP), np.float32)
        xa[0:64, 0, 0:HW] = xs[0]
        xa[64:128, 0, 0:HW] = xs[1]
        xa[64:128, 1, 0:HW] = xs[2]
        xa[0:64, 1, 0:HW] = xs[3]
        blob = blob_common.copy()
        rvs = rvec[BLOC * c : BLOC * (c + 1)]                # [4, 512]
        rvt = rvs.T.reshape(4, 128, BLOC).transpose(1, 0, 2)  # [128, 4, 4]
        blob[:, BL_RVT[0] : BL_RVT[1]] = pack_bf16(rvt)
        m = {"blobd": blob, "seld": selm, "cwpd": cwpd}
        for k, (r0, nr) in enumerate(XPIECES):
            a = r0 * W
            m[f"x{k}d"] = np.ascontiguousarray(
                xa[:, :, a : a + XPN[k]].astype(bf16)
            )
        in_maps.append(m)
    return in_maps


def kernel(**inputs):
    from concourse.bass_utils import run_bass_kernel_spmd

    nc = _get_nc()
    in_maps = make_in_maps(inputs)
    res = run_bass_kernel_spmd(nc, in_maps, core_ids=list(range(NCORES)))
    return np.concatenate(
        [np.asarray(r["out"]).astype(np.float32) for r in res.results], axis=0
    )
